# revision 1
# baseline (speedup 1.0000x reference)
"""NNCLR forward loss kernel for 8x TRN2 NeuronCores.

Strategy (hint-aligned): shard feature_queue rows across the 8 cores.
Each core computes sims = proj @ queue_shard.T for both projections
(1024 total rows) with fp32r matmuls, finds per-row shard max+argmax
(exact fp32 compare semantics, first-occurrence ties), AllGathers the
(max, argmax) pairs, selects the global winner per row, gathers the
winning queue rows by indirect DMA (owner core) + AllReduce(add), then
every core computes the 4 BxB logit matrices, log-softmax diagonals
and the final [4B] loss (replicated; host takes core 0's copy).
"""

import numpy as np

import concourse.bass as bass
import concourse.mybir as mybir
from concourse.bass import IndirectOffsetOnAxis
from concourse.tile import TileContext

import bass_rust as _br
import concourse.tile as _tile_mod


def _patched_drain_and_barrier(self, tick_clock, wait_clock):
    """Walrus here only allows 2 sem waits per instruction; split the
    Tile tail drain's wait list across extra drain instructions."""
    drain_inst = self.nc.sync.drain()
    wait_clock.add_sem_waits(
        drain_inst.ins, _br.ScopedClock({None: tick_clock.global_clock})
    )
    si = drain_inst.ins.sync_info
    if si is not None and si.on_wait and len(si.on_wait) > 1:
        waits = list(si.on_wait)
        drain_inst.ins.sync_info = _br.SyncInfo(on_wait=waits[:1], on_update=list(si.on_update))
        for i in range(1, len(waits)):
            extra = self.nc.sync.drain()
            extra.ins.sync_info = _br.SyncInfo(on_wait=waits[i : i + 1], on_update=[])
    self.nc.all_engine_barrier()
    assert self.sems is not None
    popped = self.nc._tile_sem_poison_stack.pop()
    assert popped is self._sem_poison
    self.nc.clear_and_free_semaphores(list(self.sems.allocated().values()))
    self.nc.all_engine_barrier()


_tile_mod.TileContext._drain_and_barrier = _patched_drain_and_barrier


def _split_multi_waits(nc):
    """This walrus build allows only one sync-wait per instruction; hoist
    extra waits onto NOPs inserted just before, on the same engine."""
    n_split = 0
    for f in nc.m.functions:
        for bb in f.blocks:
            il = bb.instructions
            i = 0
            while i < len(il):
                inst = il[i]
                si = inst.sync_info
                if si is not None and si.on_wait and len(si.on_wait) > 1:
                    waits = list(si.on_wait)
                    nops = []
                    for w in waits[:-1]:
                        nop = mybir.InstNoOp(
                            name=f"waitsplit-{nc.next_id()}",
                            engine=inst.engine,
                            ins=[],
                            outs=[],
                            sync_info=_br.SyncInfo(on_wait=[w], on_update=[]),
                        )
                        nc.register_instruction(nop, overwrite=True)
                        nops.append(nop)
                    inst.sync_info = _br.SyncInfo(
                        on_wait=[waits[-1]], on_update=list(si.on_update)
                    )
                    il[i:i] = nops
                    i += len(nops)
                    n_split += 1
                i += 1
    return n_split

F32 = mybir.dt.float32
F32R = mybir.dt.float32r
U16 = mybir.dt.uint16
U32 = mybir.dt.uint32

B = 512  # rows per projection
D = 256  # feature dim
B2 = 2 * B  # 1024 combined rows (p1 then p2)
NCORES = 8
Q_FULL = 98304
QS = Q_FULL // NCORES  # 12288 queue rows per core
CHUNK = 512
AF = mybir.ActivationFunctionType
ALL_CORES = [list(range(NCORES))]


def build_nc_A(qs=QS, use_f32r=False):
    """Launch A: per-core sims + exact shard max/argmax. Inputs p1T/p2T/qT."""
    nch = qs // CHUNK
    nt = B2 // 128
    nc = bass.Bass(num_devices=NCORES, debug=False)
    p1T = nc.declare_dram_parameter("p1T", [D, B], F32, isOutput=False)
    p2T = nc.declare_dram_parameter("p2T", [D, B], F32, isOutput=False)
    qT = nc.declare_dram_parameter("qT", [D, qs], F32, isOutput=False)
    mj_out = nc.declare_dram_parameter("mj", [128, 16], F32, isOutput=True)

    def mmcast(ap):
        return ap.bitcast(F32R) if use_f32r else ap

    with TileContext(nc) as tc:
        with (
            tc.tile_pool(name="persist", bufs=1) as pp,
            tc.tile_pool(name="qchunk", bufs=3) as qpool,
            tc.tile_pool(name="sims", bufs=2) as simpool,
            tc.tile_pool(name="small", bufs=4) as sp,
            tc.tile_pool(name="psumA", bufs=6, space="PSUM") as psA,
        ):
            pT_all = pp.tile([128, 2, B2], F32)
            nc.sync.dma_start(pT_all[:, :, 0:B], p1T.ap().rearrange("(k p) b -> p k b", p=128))
            nc.sync.dma_start(pT_all[:, :, B:B2], p2T.ap().rearrange("(k p) b -> p k b", p=128))

            m_all = pp.tile([128, nt], F32)
            jf_all = pp.tile([128, nt], F32)
            qT3 = qT.ap().rearrange("(k p) q -> p k q", p=128)
            for pr in range(nt // 2):
                sims_t = [
                    simpool.tile([128, qs], F32, tag="sims", name=f"sims_{pr}_{ti}")
                    for ti in range(2)
                ]
                for c in range(nch):
                    qt = qpool.tile([128, 2, CHUNK], F32)
                    nc.sync.dma_start(qt[:], qT3[:, :, c * CHUNK : (c + 1) * CHUNK])
                    for ti in range(2):
                        t = pr * 2 + ti
                        ps = psA.tile([128, CHUNK], F32)
                        nc.tensor.matmul(
                            ps[:],
                            mmcast(pT_all[:, 0, t * 128 : (t + 1) * 128]),
                            mmcast(qt[:, 0, :]),
                            start=True, stop=False,
                        )
                        nc.tensor.matmul(
                            ps[:],
                            mmcast(pT_all[:, 1, t * 128 : (t + 1) * 128]),
                            mmcast(qt[:, 1, :]),
                            start=False, stop=True,
                        )
                        nc.scalar.copy(sims_t[ti][:, c * CHUNK : (c + 1) * CHUNK], ps[:])
                for ti in range(2):
                    t = pr * 2 + ti
                    top8 = sp.tile([128, 8], F32)
                    nc.vector.max(top8[:], sims_t[ti][:])
                    idx8 = sp.tile([128, 8], U32)
                    nc.vector.max_index(idx8[:], top8[:], sims_t[ti][:])
                    nc.vector.tensor_copy(m_all[:, t : t + 1], top8[:, 0:1])
                    nc.vector.tensor_copy(jf_all[:, t : t + 1], idx8[:, 0:1])

            pack = pp.tile([128, 16], F32)
            nc.vector.tensor_copy(pack[:, 0:8], m_all[:])
            nc.vector.tensor_copy(pack[:, 8:16], jf_all[:])
            nc.sync.dma_start(mj_out.ap(), pack[:])

    _split_multi_waits(nc)
    return nc


def build_nc_C(use_f32r=False):
    """Launch C: logits + log-softmax loss from host-gathered nn rows."""
    nt = B2 // 128
    nc = bass.Bass(num_devices=NCORES, debug=False)
    p1 = nc.declare_dram_parameter("p1", [B, D], F32, isOutput=False)
    p2 = nc.declare_dram_parameter("p2", [B, D], F32, isOutput=False)
    nn_in = nc.declare_dram_parameter("nn", [128, nt, D], F32, isOutput=False)
    temp = nc.declare_dram_parameter("temp", [1, 1], F32, isOutput=False)
    loss_out = nc.declare_dram_parameter("loss", [16, 128], F32, isOutput=True)

    def mmcast(ap):
        return ap.bitcast(F32R) if use_f32r else ap

    with TileContext(nc) as tc:
        with (
            tc.tile_pool(name="persist", bufs=1) as pp,
            tc.tile_pool(name="small", bufs=2) as sp,
        ):
            p_nat = pp.tile([128, nt, D], F32)
            nc.sync.dma_start(p_nat[:, 0 : nt // 2, :], p1.ap().rearrange("(t p) d -> p t d", p=128))
            nc.sync.dma_start(p_nat[:, nt // 2 : nt, :], p2.ap().rearrange("(t p) d -> p t d", p=128))
            nn_full = pp.tile([128, nt, D], F32)
            nc.sync.dma_start(nn_full[:], nn_in.ap())

            t128 = pp.tile([128, 1], F32)
            nc.sync.dma_start(t128[:], temp.ap().to_broadcast((128, 1)))
            itb = pp.tile([128, 1], F32)
            nc.vector.reciprocal(itb[:], t128[:])

            sq = sp.tile([128, nt, D], F32, bufs=1)
            nc.vector.tensor_mul(sq[:], p_nat[:], p_nat[:])
            n2 = pp.tile([128, nt], F32)
            nc.vector.reduce_sum(n2[:], sq[:], axis=mybir.AxisListType.X)
            nrm = pp.tile([128, nt], F32)
            nc.scalar.sqrt(nrm[:], n2[:])
            nc.vector.tensor_scalar_max(nrm[:], nrm[:], 1e-12)
            inv = pp.tile([128, nt], F32)
            nc.vector.reciprocal(inv[:], nrm[:])
            invs = pp.tile([128, nt], F32)
            nc.vector.tensor_mul(invs[:], inv[:], itb[:, 0:1].to_broadcast((128, nt)))
            p_norm = pp.tile([128, nt, D], F32)
            nc.vector.tensor_mul(p_norm[:], p_nat[:], inv[:, :, None].to_broadcast((128, nt, D)))
            p_scal = pp.tile([128, nt, D], F32)
            nc.vector.tensor_mul(p_scal[:], p_nat[:], invs[:, :, None].to_broadcast((128, nt, D)))

            nn_adj = pp.tile([128, nt, D], F32)
            nc.vector.tensor_sub(nn_adj[:], nn_full[:], p_norm[:])
            nc.vector.tensor_add(nn_adj[:], p_norm[:], nn_adj[:])

            ident_dram = nc.inline_tensor(np.eye(128, dtype=np.float32), name="ident128")
            ident = pp.tile([128, 128], F32)
            nc.sync.dma_start(ident[:], ident_dram.ap())

            nnT = pp.tile([128, 2, B2], F32)
            psT = pp.tile([128, 2, B2], F32)
            with tc.tile_pool(name="psumT", bufs=4, space="PSUM") as psT_pool:
                for t in range(nt):
                    for kblk in range(2):
                        ptile = psT_pool.tile([128, 128], F32, tag="tp")
                        nc.tensor.transpose(ptile[:], nn_adj[:, t, kblk * 128 : (kblk + 1) * 128], ident[:])
                        nc.vector.tensor_copy(nnT[:, kblk, t * 128 : (t + 1) * 128], ptile[:])
                        ptile2 = psT_pool.tile([128, 128], F32, tag="tp")
                        nc.tensor.transpose(ptile2[:], p_scal[:, t, kblk * 128 : (kblk + 1) * 128], ident[:])
                        nc.scalar.copy(psT[:, kblk, t * 128 : (t + 1) * 128], ptile2[:])

            h = nt // 2
            dmul = sp.tile([128, nt, D], F32, tag="dmul", bufs=1)
            nc.vector.tensor_mul(dmul[:, 0:h, :], nn_adj[:, 0:h, :], p_scal[:, h:nt, :])
            nc.vector.tensor_mul(dmul[:, h:nt, :], nn_adj[:, h:nt, :], p_scal[:, 0:h, :])
            dg = pp.tile([128, nt], F32)
            nc.vector.reduce_sum(dg[:], dmul[:], axis=mybir.AxisListType.X)

            sl = pp.tile([128, 16], F32)
            nc.vector.tensor_copy(sl[:, 0:4], dg[:, 0:4])
            nc.vector.tensor_copy(sl[:, 4:8], dg[:, 0:4])
            nc.vector.tensor_copy(sl[:, 8:12], dg[:, 4:8])
            nc.vector.tensor_copy(sl[:, 12:16], dg[:, 4:8])

            Mall = pp.tile([128, 16], F32)
            negM = pp.tile([128, 16], F32)
            Sall = pp.tile([128, 16], F32)
            nn1T = nnT[:, :, 0:B]
            nn2T = nnT[:, :, B:B2]
            p1sT = psT[:, :, 0:B]
            p2sT = psT[:, :, B:B2]
            with tc.tile_pool(name="psumC", bufs=8, space="PSUM") as psC_pool:
                for rt in range(16):
                    mat = rt // 4
                    i = rt % 4
                    if mat == 0:
                        lhs, rhs = nn1T, p2sT
                    elif mat == 1:
                        lhs, rhs = p2sT, nn1T
                    elif mat == 2:
                        lhs, rhs = nn2T, p1sT
                    else:
                        lhs, rhs = p1sT, nn2T
                    psc = psC_pool.tile([128, B], F32)
                    for kblk in range(2):
                        nc.tensor.matmul(
                            psc[:],
                            mmcast(lhs[:, kblk, i * 128 : (i + 1) * 128]),
                            mmcast(rhs[:, kblk, :]),
                            start=(kblk == 0), stop=(kblk == 1),
                        )
                    nc.vector.reduce_max(Mall[:, rt : rt + 1], psc[:], axis=mybir.AxisListType.X)
                    nc.vector.tensor_scalar_mul(negM[:, rt : rt + 1], Mall[:, rt : rt + 1], -1.0)
                    escr = sp.tile([128, B], F32, tag="escr", bufs=2)
                    nc.scalar.activation(
                        escr[:], psc[:], AF.Exp,
                        bias=negM[:, rt : rt + 1], scale=1.0,
                        accum_out=Sall[:, rt : rt + 1],
                    )

            lnS = pp.tile([128, 16], F32)
            nc.scalar.activation(lnS[:], Sall[:], AF.Ln)
            lossT = pp.tile([128, 16], F32)
            nc.vector.tensor_add(lossT[:], lnS[:], Mall[:])
            nc.vector.tensor_sub(lossT[:], lossT[:], sl[:])
            nc.sync.dma_start(loss_out.ap().rearrange("rt p -> p rt"), lossT[:])

    _split_multi_waits(nc)
    return nc


_CACHE = {}


def _get_nc(which, use_f32r=False):
    key = (which, use_f32r)
    if key not in _CACHE:
        _CACHE[key] = build_nc_A(use_f32r=use_f32r) if which == "A" else build_nc_C(use_f32r=use_f32r)
    return _CACHE[key]


def kernel(projections_1, projections_2, feature_queue, temperature):
    from concourse.bass_utils import run_bass_kernel_spmd

    p1 = np.ascontiguousarray(projections_1, dtype=np.float32)
    p2 = np.ascontiguousarray(projections_2, dtype=np.float32)
    fq = np.ascontiguousarray(feature_queue, dtype=np.float32)
    t = np.array(temperature, dtype=np.float32).reshape(1, 1)
    p1T = np.ascontiguousarray(p1.T)
    p2T = np.ascontiguousarray(p2.T)

    # ---- launch A: sharded sims + per-core exact top-1 ----
    ncA = _get_nc("A")
    in_maps = []
    for c in range(NCORES):
        shard = fq[c * QS : (c + 1) * QS]
        in_maps.append({"p1T": p1T, "p2T": p2T, "qT": np.ascontiguousarray(shard.T)})
    resA = run_bass_kernel_spmd(ncA, in_maps, core_ids=list(range(NCORES)))
    mj = np.stack([np.asarray(resA.results[c]["mj"]) for c in range(NCORES)])  # [8, 128, 16]
    # row r = t*128 + p lives at mj[c, p, t] / mj[c, p, 8+t]
    m_g = mj[:, :, 0:8].transpose(0, 2, 1).reshape(NCORES, B2)  # [core, row]
    j_g = mj[:, :, 8:16].transpose(0, 2, 1).reshape(NCORES, B2)
    wc = np.argmax(m_g, axis=0)  # first-occurrence ties -> lowest core, matching global argmax
    jglob = wc * QS + j_g[wc, np.arange(B2)].astype(np.int64)
    nn = fq[jglob]  # [1024, 256]

    # ---- launch C: logits + loss on one core ----
    ncC = _get_nc("C")
    nn_dev = np.ascontiguousarray(nn.reshape(8, 128, D).transpose(1, 0, 2))
    resC = run_bass_kernel_spmd(
        ncC, [{"p1": p1, "p2": p2, "nn": nn_dev, "temp": t}], core_ids=[0]
    )
    loss = np.asarray(resC.results[0]["loss"], dtype=np.float32).reshape(-1)
    return loss



# revision 2
# speedup vs baseline: 3.3939x; 3.3939x over previous
"""NNCLR forward loss kernel for 8x TRN2 NeuronCores.

Launch A (8 cores): fp32r sims of both projections (1024 rows) against the
core's queue shard (12288 rows). Scan split across engines: per tile of 128
rows x 6 blocks of 2048 cols, blocks b in {1,4} get a DVE grouped max (groups
of 32, fp32 from PSUM), blocks {0,2,3,5} get Act PSUM->SBUF bf16 convert +
DVE 2x-mode positional running max. Host screens all reported maxima within
an eps that covers fp32r + bf16 error, rescores candidate positions exactly
in fp64, and picks the global top-1 per row (first-occurrence ties).

Launch C (8 cores): each core computes 2 of the 16 [128, 512] logit row-tiles
(fp32r matmuls on host-prepped transposed operands), log-softmax + diagonal
extraction, returning its [2, 128] loss slice.
"""

import numpy as np

import concourse.bass as bass
import concourse.mybir as mybir
from concourse.tile import TileContext

import bass_rust as _br
import concourse.tile as _tile_mod


def _patched_drain_and_barrier(self, tick_clock, wait_clock):
    """Walrus here only allows 2 sem waits per instruction; split the
    Tile tail drain's wait list across extra drain instructions."""
    drain_inst = self.nc.sync.drain()
    wait_clock.add_sem_waits(
        drain_inst.ins, _br.ScopedClock({None: tick_clock.global_clock})
    )
    si = drain_inst.ins.sync_info
    if si is not None and si.on_wait and len(si.on_wait) > 1:
        waits = list(si.on_wait)
        drain_inst.ins.sync_info = _br.SyncInfo(on_wait=waits[:1], on_update=list(si.on_update))
        for i in range(1, len(waits)):
            extra = self.nc.sync.drain()
            extra.ins.sync_info = _br.SyncInfo(on_wait=waits[i : i + 1], on_update=[])
    self.nc.all_engine_barrier()
    assert self.sems is not None
    popped = self.nc._tile_sem_poison_stack.pop()
    assert popped is self._sem_poison
    self.nc.clear_and_free_semaphores(list(self.sems.allocated().values()))
    self.nc.all_engine_barrier()


_tile_mod.TileContext._drain_and_barrier = _patched_drain_and_barrier


def _split_multi_waits(nc):
    """This walrus build allows only one sync-wait per instruction; hoist
    extra waits onto NOPs inserted just before, on the same engine."""
    n_split = 0
    for f in nc.m.functions:
        for bb in f.blocks:
            il = bb.instructions
            i = 0
            while i < len(il):
                inst = il[i]
                si = inst.sync_info
                if si is not None and si.on_wait and len(si.on_wait) > 1:
                    waits = list(si.on_wait)
                    nops = []
                    for w in waits[:-1]:
                        nop = mybir.InstNoOp(
                            name=f"waitsplit-{nc.next_id()}",
                            engine=inst.engine,
                            ins=[],
                            outs=[],
                            sync_info=_br.SyncInfo(on_wait=[w], on_update=[]),
                        )
                        nc.register_instruction(nop, overwrite=True)
                        nops.append(nop)
                    inst.sync_info = _br.SyncInfo(
                        on_wait=[waits[-1]], on_update=list(si.on_update)
                    )
                    il[i:i] = nops
                    i += len(nops)
                    n_split += 1
                i += 1
    return n_split


F32 = mybir.dt.float32
F32R = mybir.dt.float32r
BF16 = mybir.dt.bfloat16
AF = mybir.ActivationFunctionType
AX = mybir.AxisListType

B = 512
D = 256
B2 = 2 * B  # 1024 combined rows (p1 then p2)
NCORES = 8
Q_FULL = 98304
QS = Q_FULL // NCORES  # 12288
NT = B2 // 128  # 8 row tiles
NB = QS // 2048  # 6 col blocks of 2048
DIRECT_BS = (1, 4)  # blocks scanned by DVE grouped-max (fp32, groups of 32)
ACT_BS = (0, 2, 3, 5)  # blocks on the Act->bf16->DVE positional-max path
GROUP = 32
# fp32r abs error bound: measured 7.5e-4 max (||p||~16 rows); 3x margin
DELTA_R = 2.5e-3


def build_nc_A():
    nc = bass.Bass(num_devices=NCORES, debug=False)
    pT = nc.declare_dram_parameter("pT", [D, B2], F32R, isOutput=False)
    qT = nc.declare_dram_parameter("qT", [D, QS], F32R, isOutput=False)
    gmax_out = nc.declare_dram_parameter(
        "gmax", [128, NT, len(DIRECT_BS), 2048 // GROUP], F32, isOutput=True
    )
    accb_out = nc.declare_dram_parameter("accb", [128, NT, 2048], BF16, isOutput=True)

    with TileContext(nc) as tc:
        with (
            tc.tile_pool(name="persist", bufs=1) as pp,
            tc.tile_pool(name="bconv", bufs=3) as bp,
            tc.tile_pool(name="psumA", bufs=2, space="PSUM") as psA,
        ):
            pT_sb = pp.tile([128, 2, B2], F32R)
            nc.sync.dma_start(pT_sb[:], pT.ap().rearrange("(k p) b -> p k b", p=128))
            q_sb = pp.tile([128, 2, QS], F32R)
            qT3 = qT.ap().rearrange("(k p) q -> p k q", p=128)
            for b in range(NB):
                nc.sync.dma_start(
                    q_sb[:, :, b * 2048 : (b + 1) * 2048],
                    qT3[:, :, b * 2048 : (b + 1) * 2048],
                )
            gmax_sb = pp.tile([128, NT, len(DIRECT_BS), 2048 // GROUP], F32)
            accb_sb = pp.tile([128, NT, 2048], BF16)

            for t in range(NT):
                for b in range(NB):
                    ps = psA.tile([128, 2048], F32, tag="blk", name=f"ps_{t}_{b}")
                    for c4 in range(4):
                        col = b * 2048 + c4 * 512
                        for k in range(2):
                            nc.tensor.matmul(
                                ps[:, c4 * 512 : (c4 + 1) * 512],
                                pT_sb[:, k, t * 128 : (t + 1) * 128],
                                q_sb[:, k, col : col + 512],
                                start=(k == 0),
                                stop=(k == 1),
                            )
                    if b in DIRECT_BS:
                        slot = DIRECT_BS.index(b)
                        nc.vector.reduce_max(
                            gmax_sb[:, t, slot, :],
                            ps.rearrange("p (g k) -> p g k", k=GROUP),
                            axis=AX.X,
                        )
                    elif b == ACT_BS[0]:
                        nc.scalar.copy(accb_sb[:, t, :], ps[:])
                    else:
                        bc = bp.tile([128, 2048], BF16, tag="bc", name=f"bc_{t}_{b}")
                        nc.scalar.copy(bc[:], ps[:])
                        nc.vector.tensor_max(accb_sb[:, t, :], accb_sb[:, t, :], bc[:])
                nc.sync.dma_start(accb_out.ap()[:, t, :], accb_sb[:, t, :])
            nc.sync.dma_start(gmax_out.ap(), gmax_sb[:])

    _split_multi_waits(nc)
    return nc


def build_nc_C():
    nc = bass.Bass(num_devices=NCORES, debug=False)
    lhsT = nc.declare_dram_parameter("lhsT", [D, 256], F32R, isOutput=False)
    rhsT = nc.declare_dram_parameter("rhsT", [D, B], F32R, isOutput=False)
    lhsN = nc.declare_dram_parameter("lhsN", [128, 2, D], F32, isOutput=False)
    rhsN = nc.declare_dram_parameter("rhsN", [128, 2, D], F32, isOutput=False)
    loss_out = nc.declare_dram_parameter("loss", [2, 128], F32, isOutput=True)

    with TileContext(nc) as tc:
        with (
            tc.tile_pool(name="persist", bufs=1) as pp,
            tc.tile_pool(name="scr", bufs=2) as sp,
            tc.tile_pool(name="psumC", bufs=2, space="PSUM") as psC,
        ):
            lhsT_sb = pp.tile([128, 2, 256], F32R)
            nc.sync.dma_start(lhsT_sb[:], lhsT.ap().rearrange("(k p) c -> p k c", p=128))
            rhsT_sb = pp.tile([128, 2, B], F32R)
            nc.sync.dma_start(rhsT_sb[:], rhsT.ap().rearrange("(k p) c -> p k c", p=128))
            lhsN_sb = pp.tile([128, 2, D], F32)
            nc.sync.dma_start(lhsN_sb[:], lhsN.ap())
            rhsN_sb = pp.tile([128, 2, D], F32)
            nc.sync.dma_start(rhsN_sb[:], rhsN.ap())

            M = pp.tile([128, 2], F32)
            negM = pp.tile([128, 2], F32)
            S = pp.tile([128, 2], F32)
            for j in range(2):
                psc = psC.tile([128, B], F32, tag="psc", name=f"psc_{j}")
                for k in range(2):
                    nc.tensor.matmul(
                        psc[:],
                        lhsT_sb[:, k, j * 128 : (j + 1) * 128],
                        rhsT_sb[:, k, :],
                        start=(k == 0),
                        stop=(k == 1),
                    )
                nc.vector.reduce_max(M[:, j : j + 1], psc[:], axis=AX.X)
                nc.vector.tensor_scalar_mul(negM[:, j : j + 1], M[:, j : j + 1], -1.0)
                escr = sp.tile([128, B], F32, tag="escr", name=f"escr_{j}")
                nc.scalar.activation(
                    escr[:], psc[:], AF.Exp,
                    bias=negM[:, j : j + 1], scale=1.0,
                    accum_out=S[:, j : j + 1],
                )

            dmul = pp.tile([128, 2, D], F32)
            nc.vector.tensor_mul(dmul[:], lhsN_sb[:], rhsN_sb[:])
            dg = pp.tile([128, 2], F32)
            nc.vector.reduce_sum(dg[:], dmul[:], axis=AX.X)
            lnS = pp.tile([128, 2], F32)
            nc.scalar.activation(lnS[:], S[:], AF.Ln)
            lossT = pp.tile([128, 2], F32)
            nc.vector.tensor_add(lossT[:], lnS[:], M[:])
            nc.vector.tensor_sub(lossT[:], lossT[:], dg[:])
            nc.sync.dma_start(loss_out.ap().rearrange("j p -> p j"), lossT[:])

    _split_multi_waits(nc)
    return nc


_CACHE = {}


def _get_nc(which):
    if which not in _CACHE:
        _CACHE[which] = build_nc_A() if which == "A" else build_nc_C()
    return _CACHE[which]


def _prep_A_inmaps(p1, p2, fq):
    pT_full = np.ascontiguousarray(np.concatenate([p1, p2], axis=0).T)
    return [
        {"pT": pT_full, "qT": np.ascontiguousarray(fq[c * QS : (c + 1) * QS].T)}
        for c in range(NCORES)
    ]


def _host_top1(resA_results, p1, p2, fq):
    """Screen device maxima, rescore candidates in fp64, return jglob[1024]."""
    gm = np.stack([np.asarray(resA_results[c]["gmax"]) for c in range(NCORES)])
    ab = np.stack(
        [np.asarray(resA_results[c]["accb"]).astype(np.float32) for c in range(NCORES)]
    )
    # [c, 128p, t, ...] -> row r = t*128 + p
    gm = gm.transpose(0, 2, 1, 3, 4).reshape(NCORES, B2, len(DIRECT_BS), 2048 // GROUP)
    ab = ab.transpose(0, 2, 1, 3).reshape(NCORES, B2, 2048)

    Mt = np.maximum(gm.max(axis=(0, 2, 3)), ab.max(axis=(0, 2)))  # [B2]
    eps = 2 * DELTA_R + np.abs(Mt) * 2.0 ** -8 + 1e-4
    thresh = Mt - eps

    rows_list, pos_list = [], []
    c_i, r_i, s_i, g_i = np.nonzero(gm >= thresh[None, :, None, None])
    if len(c_i):
        base = (
            c_i.astype(np.int64) * QS
            + np.asarray(DIRECT_BS, dtype=np.int64)[s_i] * 2048
            + g_i.astype(np.int64) * GROUP
        )
        pos = (base[:, None] + np.arange(GROUP, dtype=np.int64)[None, :]).reshape(-1)
        rows = np.repeat(r_i.astype(np.int64), GROUP)
        rows_list.append(rows)
        pos_list.append(pos)
    c_i, r_i, p_i = np.nonzero(ab >= thresh[None, :, None])
    if len(c_i):
        act_b = np.asarray(ACT_BS, dtype=np.int64)
        base = c_i.astype(np.int64) * QS + p_i.astype(np.int64)
        pos = (base[:, None] + act_b[None, :] * 2048).reshape(-1)
        rows = np.repeat(r_i.astype(np.int64), len(ACT_BS))
        rows_list.append(rows)
        pos_list.append(pos)
    rows = np.concatenate(rows_list)
    pos = np.concatenate(pos_list)

    P64 = np.concatenate([p1, p2], axis=0).astype(np.float64)
    s = np.einsum("kd,kd->k", fq[pos].astype(np.float64), P64[rows])

    # first-occurrence argmax per row: sort by (row, pos), take first pos
    # attaining the row max
    order = np.lexsort((pos, rows))
    rows_s, pos_s, s_s = rows[order], pos[order], s[order]
    jglob = np.empty(B2, dtype=np.int64)
    starts = np.searchsorted(rows_s, np.arange(B2), side="left")
    ends = np.searchsorted(rows_s, np.arange(B2), side="right")
    for r in range(B2):
        sl = slice(starts[r], ends[r])
        sv = s_s[sl]
        jglob[r] = pos_s[sl][np.argmax(sv)]
    return jglob


def _prep_C_inmaps(p1, p2, nn, temp):
    def l2n(x):
        n = np.sqrt((x.astype(np.float64) ** 2).sum(axis=1, keepdims=True))
        return (x / np.maximum(n, 1e-12)).astype(np.float32)

    p1n = l2n(p1)
    p2n = l2n(p2)
    inv_t = np.float32(1.0) / np.float32(temp)
    p1s = (p1n * inv_t).astype(np.float32)
    p2s = (p2n * inv_t).astype(np.float32)
    nn1, nn2 = nn[:B], nn[B:]
    nn1_adj = ((nn1 - p1n) + p1n).astype(np.float32)
    nn2_adj = ((nn2 - p2n) + p2n).astype(np.float32)

    mats = [(nn1_adj, p2s), (p2s, nn1_adj), (nn2_adj, p1s), (p1s, nn2_adj)]
    in_maps = []
    for c in range(NCORES):
        m = c // 2
        i0 = (c % 2) * 2
        lhs, rhs = mats[m]
        lhsT = np.ascontiguousarray(lhs.T[:, i0 * 128 : (i0 + 2) * 128])
        rhsT = np.ascontiguousarray(rhs.T)
        lhsN = np.ascontiguousarray(
            lhs.reshape(4, 128, D)[i0 : i0 + 2].transpose(1, 0, 2)
        )
        rhsN = np.ascontiguousarray(
            rhs.reshape(4, 128, D)[i0 : i0 + 2].transpose(1, 0, 2)
        )
        in_maps.append({"lhsT": lhsT, "rhsT": rhsT, "lhsN": lhsN, "rhsN": rhsN})
    return in_maps


def kernel(projections_1, projections_2, feature_queue, temperature):
    from concourse.bass_utils import run_bass_kernel_spmd

    p1 = np.ascontiguousarray(projections_1, dtype=np.float32)
    p2 = np.ascontiguousarray(projections_2, dtype=np.float32)
    fq = np.ascontiguousarray(feature_queue, dtype=np.float32)

    ncA = _get_nc("A")
    resA = run_bass_kernel_spmd(ncA, _prep_A_inmaps(p1, p2, fq), core_ids=list(range(NCORES)))
    jglob = _host_top1(resA.results, p1, p2, fq)
    nn = fq[jglob]

    ncC = _get_nc("C")
    resC = run_bass_kernel_spmd(
        ncC, _prep_C_inmaps(p1, p2, nn, temperature), core_ids=list(range(NCORES))
    )
    loss = np.empty(4 * B, dtype=np.float32)
    for c in range(NCORES):
        out = np.asarray(resC.results[c]["loss"], dtype=np.float32)
        for j in range(2):
            rt = 2 * c + j
            loss[rt * 128 : (rt + 1) * 128] = out[j]
    return loss


# revision 6
# speedup vs baseline: 3.5948x; 1.0592x over previous
"""NNCLR forward loss kernel for 8x TRN2 NeuronCores.

Launch A (8 cores): fp32r sims of both projections (1024 rows) against the
core's queue shard (12288 rows). Scan split across engines: per tile of 128
rows x 6 blocks of 2048 cols, blocks b in {1,4} get a DVE grouped max (groups
of 32, fp32 from PSUM), blocks {0,2,3,5} get Act PSUM->SBUF bf16 convert +
DVE 2x-mode positional running max. Host screens all reported maxima within
an eps that covers fp32r + bf16 error, rescores candidate positions exactly
in fp64, and picks the global top-1 per row (first-occurrence ties).

Launch C (8 cores): each core computes 2 of the 16 [128, 512] logit row-tiles
(fp32r matmuls on host-prepped transposed operands), log-softmax + diagonal
extraction, returning its [2, 128] loss slice.
"""

import numpy as np

import concourse.bass as bass
import concourse.mybir as mybir
from concourse.tile import TileContext

import bass_rust as _br
import concourse.tile as _tile_mod


def _patched_drain_and_barrier(self, tick_clock, wait_clock):
    """Walrus here only allows 2 sem waits per instruction; split the
    Tile tail drain's wait list across extra drain instructions."""
    drain_inst = self.nc.sync.drain()
    wait_clock.add_sem_waits(
        drain_inst.ins, _br.ScopedClock({None: tick_clock.global_clock})
    )
    si = drain_inst.ins.sync_info
    if si is not None and si.on_wait and len(si.on_wait) > 1:
        waits = list(si.on_wait)
        drain_inst.ins.sync_info = _br.SyncInfo(on_wait=waits[:1], on_update=list(si.on_update))
        for i in range(1, len(waits)):
            extra = self.nc.sync.drain()
            extra.ins.sync_info = _br.SyncInfo(on_wait=waits[i : i + 1], on_update=[])
    self.nc.all_engine_barrier()
    assert self.sems is not None
    popped = self.nc._tile_sem_poison_stack.pop()
    assert popped is self._sem_poison
    self.nc.clear_and_free_semaphores(list(self.sems.allocated().values()))
    self.nc.all_engine_barrier()


_tile_mod.TileContext._drain_and_barrier = _patched_drain_and_barrier


def _split_multi_waits(nc):
    """This walrus build allows only one sync-wait per instruction; hoist
    extra waits onto NOPs inserted just before, on the same engine."""
    n_split = 0
    for f in nc.m.functions:
        for bb in f.blocks:
            il = bb.instructions
            i = 0
            while i < len(il):
                inst = il[i]
                si = inst.sync_info
                if si is not None and si.on_wait and len(si.on_wait) > 1:
                    waits = list(si.on_wait)
                    nops = []
                    for w in waits[:-1]:
                        nop = mybir.InstNoOp(
                            name=f"waitsplit-{nc.next_id()}",
                            engine=inst.engine,
                            ins=[],
                            outs=[],
                            sync_info=_br.SyncInfo(on_wait=[w], on_update=[]),
                        )
                        nc.register_instruction(nop, overwrite=True)
                        nops.append(nop)
                    inst.sync_info = _br.SyncInfo(
                        on_wait=[waits[-1]], on_update=list(si.on_update)
                    )
                    il[i:i] = nops
                    i += len(nops)
                    n_split += 1
                i += 1
    return n_split


F32 = mybir.dt.float32
F32R = mybir.dt.float32r
BF16 = mybir.dt.bfloat16
AF = mybir.ActivationFunctionType
AX = mybir.AxisListType

B = 512
D = 256
B2 = 2 * B  # 1024 combined rows (p1 then p2)
NCORES = 8
Q_FULL = 98304
QS = Q_FULL // NCORES  # 12288
NT = B2 // 128  # 8 row tiles
NB = QS // 2048  # 6 col blocks of 2048
# Per tile t: blocks with (t+b)%3==0 get the DVE grouped-max (fp32 from PSUM);
# the rest go Act->bf16->DVE positional running max. 2 direct + 4 act per tile,
# evenly mixed within every block round (b outer, t inner).
DIRECT_BS = [[b for b in range(NB) if (t + b) % 3 == 0] for t in range(NT)]
ACT_BS = [[b for b in range(NB) if (t + b) % 3 != 0] for t in range(NT)]
GROUP = 32
# fp32r abs error bound: measured 7.5e-4 max (||p||~16 rows); 3x margin
DELTA_R = 2.5e-3


def build_nc_A():
    nc = bass.Bass(num_devices=NCORES, debug=False)
    pT = nc.declare_dram_parameter("pT", [D, B2], F32R, isOutput=False)
    qT = nc.declare_dram_parameter("qT", [D, QS], F32R, isOutput=False)
    gmax_out = nc.declare_dram_parameter(
        "gmax", [128, NT, 2, 2048 // GROUP], F32, isOutput=True
    )
    accb_out = nc.declare_dram_parameter("accb", [128, NT, 2048], BF16, isOutput=True)

    with TileContext(nc) as tc:
        with (
            tc.tile_pool(name="persist", bufs=1) as pp,
            tc.tile_pool(name="qstream", bufs=3) as qp,
            tc.tile_pool(name="bconv", bufs=3) as bp,
            tc.tile_pool(name="psumA", bufs=2, space="PSUM") as psA,
        ):
            pT_sb = pp.tile([128, 2, B2], F32R)
            nc.sync.dma_start(pT_sb[:], pT.ap().rearrange("(k p) b -> p k b", p=128))
            qT3 = qT.ap().rearrange("(k p) q -> p k q", p=128)
            gmax_sb = pp.tile([128, NT, 2, 2048 // GROUP], F32)
            accb_sb = pp.tile([128, NT, 2048], BF16)

            q_tiles = []
            for b in range(NB):
                qt = qp.tile([128, 2, 2048], F32R, tag="q", name=f"q_{b}")
                if b == 0:
                    # first block in 4 chunk-DMAs so matmul 0 starts ASAP
                    for c4 in range(4):
                        nc.sync.dma_start(
                            qt[:, :, c4 * 512 : (c4 + 1) * 512],
                            qT3[:, :, c4 * 512 : (c4 + 1) * 512],
                        )
                else:
                    nc.sync.dma_start(qt[:], qT3[:, :, b * 2048 : (b + 1) * 2048])
                q_tiles.append(qt)

            for b in range(NB):
                qt = q_tiles[b]
                for t in range(NT):
                    ps = psA.tile([128, 2048], F32, tag="blk", name=f"ps_{t}_{b}")
                    for k in range(2):
                        for c4 in range(4):
                            nc.tensor.matmul(
                                ps[:, c4 * 512 : (c4 + 1) * 512],
                                pT_sb[:, k, t * 128 : (t + 1) * 128],
                                qt[:, k, c4 * 512 : (c4 + 1) * 512],
                                start=(k == 0),
                                stop=(k == 1),
                            )
                    if b in DIRECT_BS[t]:
                        slot = DIRECT_BS[t].index(b)
                        nc.vector.reduce_max(
                            gmax_sb[:, t, slot, :],
                            ps.rearrange("p (g k) -> p g k", k=GROUP),
                            axis=AX.X,
                        )
                    elif b == ACT_BS[t][0]:
                        nc.scalar.copy(accb_sb[:, t, :], ps[:])
                    else:
                        bc = bp.tile([128, 2048], BF16, tag="bc", name=f"bc_{t}_{b}")
                        nc.scalar.copy(bc[:], ps[:])
                        nc.vector.tensor_max(accb_sb[:, t, :], accb_sb[:, t, :], bc[:])
                    if b == ACT_BS[t][-1]:
                        nc.sync.dma_start(accb_out.ap()[:, t, :], accb_sb[:, t, :])
                    if b == DIRECT_BS[t][-1]:
                        nc.sync.dma_start(
                            gmax_out.ap()[:, t, :, :], gmax_sb[:, t, :, :]
                        )

    _split_multi_waits(nc)
    return nc


def build_nc_C():
    nc = bass.Bass(num_devices=NCORES, debug=False)
    lhsT = nc.declare_dram_parameter("lhsT", [D, 256], F32R, isOutput=False)
    rhsT = nc.declare_dram_parameter("rhsT", [D, B], F32R, isOutput=False)
    lhsN = nc.declare_dram_parameter("lhsN", [128, 2, D], F32, isOutput=False)
    rhsN = nc.declare_dram_parameter("rhsN", [128, 2, D], F32, isOutput=False)
    loss_out = nc.declare_dram_parameter("loss", [2, 128], F32, isOutput=True)

    with TileContext(nc) as tc:
        with (
            tc.tile_pool(name="persist", bufs=1) as pp,
            tc.tile_pool(name="scr", bufs=2) as sp,
            tc.tile_pool(name="psumC", bufs=2, space="PSUM") as psC,
        ):
            rhsT3 = rhsT.ap().rearrange("(k p) c -> p k c", p=128)
            lhsT_sb = pp.tile([128, 2, 256], F32R)
            rhsT_sb = pp.tile([128, 2, B], F32R)
            # k=0 halves first (unblock the first matmul), spread across the
            # two HWDGE-capable queues (sync + scalar)
            nc.sync.dma_start(rhsT_sb[:, 0, :], rhsT3[:, 0, :])
            nc.scalar.dma_start(lhsT_sb[:], lhsT.ap().rearrange("(k p) c -> p k c", p=128))
            nc.sync.dma_start(rhsT_sb[:, 1, :], rhsT3[:, 1, :])
            lhsN_sb = pp.tile([128, 2, D], F32)
            nc.scalar.dma_start(lhsN_sb[:], lhsN.ap())
            rhsN_sb = pp.tile([128, 2, D], F32)
            nc.sync.dma_start(rhsN_sb[:], rhsN.ap())

            M = pp.tile([128, 2], F32)
            negM = pp.tile([128, 2], F32)
            S = pp.tile([128, 2], F32)
            dmul = pp.tile([128, 2, D], F32)
            dg = pp.tile([128, 2], F32)
            for j in range(2):
                psc = psC.tile([128, B], F32, tag="psc", name=f"psc_{j}")
                for k in range(2):
                    nc.tensor.matmul(
                        psc[:],
                        lhsT_sb[:, k, j * 128 : (j + 1) * 128],
                        rhsT_sb[:, k, :],
                        start=(k == 0),
                        stop=(k == 1),
                    )
                nc.vector.reduce_max(M[:, j : j + 1], psc[:], axis=AX.X)
                nc.vector.tensor_scalar_mul(negM[:, j : j + 1], M[:, j : j + 1], -1.0)
                escr = sp.tile([128, B], F32, tag="escr", name=f"escr_{j}")
                nc.scalar.activation(
                    escr[:], psc[:], AF.Exp,
                    bias=negM[:, j : j + 1], scale=1.0,
                    accum_out=S[:, j : j + 1],
                )
                # diag dot products overlap with the matmul/softmax chain
                nc.vector.tensor_mul(dmul[:, j, :], lhsN_sb[:, j, :], rhsN_sb[:, j, :])
                nc.vector.reduce_sum(dg[:, j : j + 1], dmul[:, j, :], axis=AX.X)
            lnS = pp.tile([128, 2], F32)
            nc.scalar.activation(lnS[:], S[:], AF.Ln)
            lossT = pp.tile([128, 2], F32)
            nc.vector.tensor_add(lossT[:], lnS[:], M[:])
            nc.vector.tensor_sub(lossT[:], lossT[:], dg[:])
            nc.sync.dma_start(loss_out.ap().rearrange("j p -> p j"), lossT[:])

    _split_multi_waits(nc)
    return nc


_CACHE = {}


def _get_nc(which):
    if which not in _CACHE:
        _CACHE[which] = build_nc_A() if which == "A" else build_nc_C()
    return _CACHE[which]


def _prep_A_inmaps(p1, p2, fq):
    pT_full = np.ascontiguousarray(np.concatenate([p1, p2], axis=0).T)
    return [
        {"pT": pT_full, "qT": np.ascontiguousarray(fq[c * QS : (c + 1) * QS].T)}
        for c in range(NCORES)
    ]


def _host_top1(resA_results, p1, p2, fq):
    """Screen device maxima, rescore candidates in fp64, return jglob[1024]."""
    gm = np.stack([np.asarray(resA_results[c]["gmax"]) for c in range(NCORES)])
    ab = np.stack(
        [np.asarray(resA_results[c]["accb"]).astype(np.float32) for c in range(NCORES)]
    )
    # [c, 128p, t, ...] -> row r = t*128 + p
    gm = gm.transpose(0, 2, 1, 3, 4).reshape(NCORES, B2, 2, 2048 // GROUP)
    ab = ab.transpose(0, 2, 1, 3).reshape(NCORES, B2, 2048)

    Mt = np.maximum(gm.max(axis=(0, 2, 3)), ab.max(axis=(0, 2)))  # [B2]
    eps = 2 * DELTA_R + np.abs(Mt) * 2.0 ** -8 + 1e-4
    thresh = Mt - eps

    dir_b = np.asarray(DIRECT_BS, dtype=np.int64)  # [NT, 2]
    act_b = np.asarray(ACT_BS, dtype=np.int64)  # [NT, 4]
    rows_list, pos_list = [], []
    c_i, r_i, s_i, g_i = np.nonzero(gm >= thresh[None, :, None, None])
    if len(c_i):
        base = (
            c_i.astype(np.int64) * QS
            + dir_b[r_i // 128, s_i] * 2048
            + g_i.astype(np.int64) * GROUP
        )
        pos = (base[:, None] + np.arange(GROUP, dtype=np.int64)[None, :]).reshape(-1)
        rows = np.repeat(r_i.astype(np.int64), GROUP)
        rows_list.append(rows)
        pos_list.append(pos)
    c_i, r_i, p_i = np.nonzero(ab >= thresh[None, :, None])
    if len(c_i):
        base = c_i.astype(np.int64) * QS + p_i.astype(np.int64)
        pos = (base[:, None] + act_b[r_i // 128] * 2048).reshape(-1)
        rows = np.repeat(r_i.astype(np.int64), act_b.shape[1])
        rows_list.append(rows)
        pos_list.append(pos)
    rows = np.concatenate(rows_list)
    pos = np.concatenate(pos_list)

    P64 = np.concatenate([p1, p2], axis=0).astype(np.float64)
    s = np.einsum("kd,kd->k", fq[pos].astype(np.float64), P64[rows])

    # first-occurrence argmax per row: sort by (row, pos), take first pos
    # attaining the row max
    order = np.lexsort((pos, rows))
    rows_s, pos_s, s_s = rows[order], pos[order], s[order]
    jglob = np.empty(B2, dtype=np.int64)
    starts = np.searchsorted(rows_s, np.arange(B2), side="left")
    ends = np.searchsorted(rows_s, np.arange(B2), side="right")
    for r in range(B2):
        sl = slice(starts[r], ends[r])
        sv = s_s[sl]
        jglob[r] = pos_s[sl][np.argmax(sv)]
    return jglob


def _prep_C_inmaps(p1, p2, nn, temp):
    def l2n(x):
        n = np.sqrt((x.astype(np.float64) ** 2).sum(axis=1, keepdims=True))
        return (x / np.maximum(n, 1e-12)).astype(np.float32)

    p1n = l2n(p1)
    p2n = l2n(p2)
    inv_t = np.float32(1.0) / np.float32(temp)
    p1s = (p1n * inv_t).astype(np.float32)
    p2s = (p2n * inv_t).astype(np.float32)
    nn1, nn2 = nn[:B], nn[B:]
    nn1_adj = ((nn1 - p1n) + p1n).astype(np.float32)
    nn2_adj = ((nn2 - p2n) + p2n).astype(np.float32)

    mats = [(nn1_adj, p2s), (p2s, nn1_adj), (nn2_adj, p1s), (p1s, nn2_adj)]
    in_maps = []
    for c in range(NCORES):
        m = c // 2
        i0 = (c % 2) * 2
        lhs, rhs = mats[m]
        lhsT = np.ascontiguousarray(lhs.T[:, i0 * 128 : (i0 + 2) * 128])
        rhsT = np.ascontiguousarray(rhs.T)
        lhsN = np.ascontiguousarray(
            lhs.reshape(4, 128, D)[i0 : i0 + 2].transpose(1, 0, 2)
        )
        rhsN = np.ascontiguousarray(
            rhs.reshape(4, 128, D)[i0 : i0 + 2].transpose(1, 0, 2)
        )
        in_maps.append({"lhsT": lhsT, "rhsT": rhsT, "lhsN": lhsN, "rhsN": rhsN})
    return in_maps


def kernel(projections_1, projections_2, feature_queue, temperature):
    from concourse.bass_utils import run_bass_kernel_spmd

    p1 = np.ascontiguousarray(projections_1, dtype=np.float32)
    p2 = np.ascontiguousarray(projections_2, dtype=np.float32)
    fq = np.ascontiguousarray(feature_queue, dtype=np.float32)

    ncA = _get_nc("A")
    resA = run_bass_kernel_spmd(ncA, _prep_A_inmaps(p1, p2, fq), core_ids=list(range(NCORES)))
    jglob = _host_top1(resA.results, p1, p2, fq)
    nn = fq[jglob]

    ncC = _get_nc("C")
    resC = run_bass_kernel_spmd(
        ncC, _prep_C_inmaps(p1, p2, nn, temperature), core_ids=list(range(NCORES))
    )
    loss = np.empty(4 * B, dtype=np.float32)
    for c in range(NCORES):
        out = np.asarray(resC.results[c]["loss"], dtype=np.float32)
        for j in range(2):
            rt = 2 * c + j
            loss[rt * 128 : (rt + 1) * 128] = out[j]
    return loss


# revision 12
# speedup vs baseline: 3.6642x; 1.0193x over previous
"""NNCLR forward loss kernel for 8x TRN2 NeuronCores.

Launch A (8 cores): fp32r sims of both projections (1024 rows) against the
core's queue shard (12288 rows). Scan split across engines: per tile of 128
rows x 6 blocks of 2048 cols, blocks b in {1,4} get a DVE grouped max (groups
of 32, fp32 from PSUM), blocks {0,2,3,5} get Act PSUM->SBUF bf16 convert +
DVE 2x-mode positional running max. Host screens all reported maxima within
an eps that covers fp32r + bf16 error, rescores candidate positions exactly
in fp64, and picks the global top-1 per row (first-occurrence ties).

Launch C (8 cores): each core computes 2 of the 16 [128, 512] logit row-tiles
(fp32r matmuls on host-prepped transposed operands), log-softmax + diagonal
extraction, returning its [2, 128] loss slice.
"""

import numpy as np

import concourse.bass as bass
import concourse.mybir as mybir
from concourse.tile import TileContext

import bass_rust as _br
import concourse.tile as _tile_mod


def _patched_drain_and_barrier(self, tick_clock, wait_clock):
    """Walrus here only allows 2 sem waits per instruction; split the
    Tile tail drain's wait list across extra drain instructions."""
    drain_inst = self.nc.sync.drain()
    wait_clock.add_sem_waits(
        drain_inst.ins, _br.ScopedClock({None: tick_clock.global_clock})
    )
    si = drain_inst.ins.sync_info
    if si is not None and si.on_wait and len(si.on_wait) > 1:
        waits = list(si.on_wait)
        drain_inst.ins.sync_info = _br.SyncInfo(on_wait=waits[:1], on_update=list(si.on_update))
        for i in range(1, len(waits)):
            extra = self.nc.sync.drain()
            extra.ins.sync_info = _br.SyncInfo(on_wait=waits[i : i + 1], on_update=[])
    self.nc.all_engine_barrier()
    assert self.sems is not None
    popped = self.nc._tile_sem_poison_stack.pop()
    assert popped is self._sem_poison
    self.nc.clear_and_free_semaphores(list(self.sems.allocated().values()))
    self.nc.all_engine_barrier()


_tile_mod.TileContext._drain_and_barrier = _patched_drain_and_barrier


def _split_multi_waits(nc):
    """This walrus build allows only one sync-wait per instruction; hoist
    extra waits onto NOPs inserted just before, on the same engine."""
    n_split = 0
    for f in nc.m.functions:
        for bb in f.blocks:
            il = bb.instructions
            i = 0
            while i < len(il):
                inst = il[i]
                si = inst.sync_info
                if si is not None and si.on_wait and len(si.on_wait) > 1:
                    waits = list(si.on_wait)
                    nops = []
                    for w in waits[:-1]:
                        nop = mybir.InstNoOp(
                            name=f"waitsplit-{nc.next_id()}",
                            engine=inst.engine,
                            ins=[],
                            outs=[],
                            sync_info=_br.SyncInfo(on_wait=[w], on_update=[]),
                        )
                        nc.register_instruction(nop, overwrite=True)
                        nops.append(nop)
                    inst.sync_info = _br.SyncInfo(
                        on_wait=[waits[-1]], on_update=list(si.on_update)
                    )
                    il[i:i] = nops
                    i += len(nops)
                    n_split += 1
                i += 1
    return n_split


F32 = mybir.dt.float32
F32R = mybir.dt.float32r
BF16 = mybir.dt.bfloat16
AF = mybir.ActivationFunctionType
AX = mybir.AxisListType

B = 512
D = 256
B2 = 2 * B  # 1024 combined rows (p1 then p2)
NCORES = 8
Q_FULL = 98304
QS = Q_FULL // NCORES  # 12288
NT = B2 // 128  # 8 row tiles
NB = QS // 2048  # 6 col blocks of 2048
# Per tile t: 2 blocks get the DVE grouped-max (fp32 from PSUM); the other 4
# go Act->bf16->DVE positional running max. Hand-balanced so each block round
# has ~3 direct blocks and the final round (b=5) has just one, early in the
# round, keeping the launch tail off the slow DVE reduce.
DIRECT_BS = [[2, 5], [0, 3], [1, 4], [0, 2], [1, 3], [2, 4], [0, 3], [1, 4]]
ACT_BS = [[b for b in range(NB) if b not in DIRECT_BS[t]] for t in range(NT)]
GROUP = 32
# fp32r abs error bound: measured 7.5e-4 max (||p||~16 rows); 3x margin
DELTA_R = 2.5e-3


def build_nc_A():
    nc = bass.Bass(num_devices=NCORES, debug=False)
    pT = nc.declare_dram_parameter("pT", [D, B2], F32R, isOutput=False)
    qT = nc.declare_dram_parameter("qT", [D, QS], F32R, isOutput=False)
    gmax_out = nc.declare_dram_parameter(
        "gmax", [128, NT, 2, 2048 // GROUP], F32, isOutput=True
    )
    accb_out = nc.declare_dram_parameter("accb", [128, NT, 2048], BF16, isOutput=True)

    with TileContext(nc) as tc:
        with (
            tc.tile_pool(name="persist", bufs=1) as pp,
            tc.tile_pool(name="qstream", bufs=3) as qp,
            tc.tile_pool(name="bconv", bufs=3) as bp,
            tc.tile_pool(name="psumA", bufs=2, space="PSUM") as psA,
        ):
            pT_sb = pp.tile([128, 2, B2], F32R)
            pT3 = pT.ap().rearrange("(k p) b -> p k b", p=128)
            # tiny first pieces on the scalar queue unblock warmup + matmul 0
            nc.scalar.dma_start(pT_sb[:, 0, 0:512], pT3[:, 0, 0:512])
            nc.scalar.dma_start(pT_sb[:, 1, 0:128], pT3[:, 1, 0:128])
            nc.scalar.dma_start(pT_sb[:, 0, 512:B2], pT3[:, 0, 512:B2])
            nc.scalar.dma_start(pT_sb[:, 1, 128:B2], pT3[:, 1, 128:B2])
            qT3 = qT.ap().rearrange("(k p) q -> p k q", p=128)
            gmax_sb = pp.tile([128, NT, 2, 2048 // GROUP], F32)
            accb_sb = pp.tile([128, NT, 2048], BF16)

            q_tiles = []
            for b in range(NB):
                qt = qp.tile([128, 2, 2048], F32R, tag="q", name=f"q_{b}")
                if b == 0:
                    # first block in 4 chunk-DMAs so matmul 0 starts ASAP
                    for c4 in range(4):
                        nc.sync.dma_start(
                            qt[:, :, c4 * 512 : (c4 + 1) * 512],
                            qT3[:, :, c4 * 512 : (c4 + 1) * 512],
                        )
                else:
                    nc.sync.dma_start(qt[:], qT3[:, :, b * 2048 : (b + 1) * 2048])
                q_tiles.append(qt)

            # PE p-state warmup: dummy matmuls on the pT tile into the first
            # psum block while q block 0 is still in flight. Overwritten by
            # the real (start=True) matmuls that follow on the in-order PE.
            ps00 = psA.tile([128, 2048], F32, tag="blk", name="ps_0_0")
            for w in range(8):
                nc.tensor.matmul(
                    ps00[:, 0:512],
                    pT_sb[:, 0, 0:128],
                    pT_sb[:, 0, 0:512],
                    start=True,
                    stop=True,
                    skip_group_check=True,
                )

            for b in range(NB):
                qt = q_tiles[b]
                for t in range(NT):
                    if b == 0 and t == 0:
                        ps = ps00
                    else:
                        ps = psA.tile([128, 2048], F32, tag="blk", name=f"ps_{t}_{b}")
                    for k in range(2):
                        for c4 in range(4):
                            nc.tensor.matmul(
                                ps[:, c4 * 512 : (c4 + 1) * 512],
                                pT_sb[:, k, t * 128 : (t + 1) * 128],
                                qt[:, k, c4 * 512 : (c4 + 1) * 512],
                                start=(k == 0),
                                stop=(k == 1),
                            )
                    if b in DIRECT_BS[t]:
                        slot = DIRECT_BS[t].index(b)
                        nc.vector.reduce_max(
                            gmax_sb[:, t, slot, :],
                            ps.rearrange("p (g k) -> p g k", k=GROUP),
                            axis=AX.X,
                        )
                    elif b == ACT_BS[t][0]:
                        nc.scalar.copy(accb_sb[:, t, :], ps[:])
                    else:
                        bc = bp.tile([128, 2048], BF16, tag="bc", name=f"bc_{t}_{b}")
                        nc.scalar.copy(bc[:], ps[:])
                        nc.vector.tensor_max(accb_sb[:, t, :], accb_sb[:, t, :], bc[:])
                    if b == ACT_BS[t][-1]:
                        nc.sync.dma_start(accb_out.ap()[:, t, :], accb_sb[:, t, :])
                    if b == DIRECT_BS[t][-1]:
                        nc.sync.dma_start(
                            gmax_out.ap()[:, t, :, :], gmax_sb[:, t, :, :]
                        )

    _split_multi_waits(nc)
    return nc


def build_nc_C():
    nc = bass.Bass(num_devices=NCORES, debug=False)
    lhsT = nc.declare_dram_parameter("lhsT", [D, 256], F32R, isOutput=False)
    rhsT = nc.declare_dram_parameter("rhsT", [D, B], F32R, isOutput=False)
    lhsN = nc.declare_dram_parameter("lhsN", [128, 2, D], F32, isOutput=False)
    rhsN = nc.declare_dram_parameter("rhsN", [128, 2, D], F32, isOutput=False)
    loss_out = nc.declare_dram_parameter("loss", [128, 2], F32, isOutput=True)

    with TileContext(nc) as tc:
        with (
            tc.tile_pool(name="persist", bufs=1) as pp,
            tc.tile_pool(name="scr", bufs=2) as sp,
            tc.tile_pool(name="psumC", bufs=2, space="PSUM") as psC,
        ):
            rhsT3 = rhsT.ap().rearrange("(k p) c -> p k c", p=128)
            lhsT_sb = pp.tile([128, 2, 256], F32R)
            rhsT_sb = pp.tile([128, 2, B], F32R)
            # k=0 halves first (unblock the first matmul), spread across the
            # two HWDGE-capable queues (sync + scalar)
            nc.sync.dma_start(rhsT_sb[:, 0, :], rhsT3[:, 0, :])
            nc.scalar.dma_start(lhsT_sb[:], lhsT.ap().rearrange("(k p) c -> p k c", p=128))
            nc.sync.dma_start(rhsT_sb[:, 1, :], rhsT3[:, 1, :])
            lhsN_sb = pp.tile([128, 2, D], F32)
            nc.scalar.dma_start(lhsN_sb[:], lhsN.ap())
            rhsN_sb = pp.tile([128, 2, D], F32)
            nc.sync.dma_start(rhsN_sb[:], rhsN.ap())

            M = pp.tile([128, 2], F32)
            negM = pp.tile([128, 2], F32)
            S = pp.tile([128, 2], F32)
            dmul = pp.tile([128, 2, D], F32)
            dg = pp.tile([128, 2], F32)
            for j in range(2):
                psc = psC.tile([128, B], F32, tag="psc", name=f"psc_{j}")
                for k in range(2):
                    nc.tensor.matmul(
                        psc[:],
                        lhsT_sb[:, k, j * 128 : (j + 1) * 128],
                        rhsT_sb[:, k, :],
                        start=(k == 0),
                        stop=(k == 1),
                    )
                nc.vector.reduce_max(M[:, j : j + 1], psc[:], axis=AX.X)
                nc.vector.tensor_scalar_mul(negM[:, j : j + 1], M[:, j : j + 1], -1.0)
                escr = sp.tile([128, B], F32, tag="escr", name=f"escr_{j}")
                nc.scalar.activation(
                    escr[:], psc[:], AF.Exp,
                    bias=negM[:, j : j + 1], scale=1.0,
                    accum_out=S[:, j : j + 1],
                )
                # diag dot products overlap with the matmul/softmax chain
                nc.vector.tensor_mul(dmul[:, j, :], lhsN_sb[:, j, :], rhsN_sb[:, j, :])
                nc.vector.reduce_sum(dg[:, j : j + 1], dmul[:, j, :], axis=AX.X)
            lnS = pp.tile([128, 2], F32)
            nc.scalar.activation(lnS[:], S[:], AF.Ln)
            lossT = pp.tile([128, 2], F32)
            nc.vector.tensor_add(lossT[:], lnS[:], M[:])
            nc.vector.tensor_sub(lossT[:], lossT[:], dg[:])
            nc.sync.dma_start(loss_out.ap(), lossT[:])

    _split_multi_waits(nc)
    return nc


_CACHE = {}


def _get_nc(which):
    if which not in _CACHE:
        _CACHE[which] = build_nc_A() if which == "A" else build_nc_C()
    return _CACHE[which]


def _prep_A_inmaps(p1, p2, fq):
    pT_full = np.ascontiguousarray(np.concatenate([p1, p2], axis=0).T)
    return [
        {"pT": pT_full, "qT": np.ascontiguousarray(fq[c * QS : (c + 1) * QS].T)}
        for c in range(NCORES)
    ]


def _host_top1(resA_results, p1, p2, fq):
    """Screen device maxima, rescore candidates in fp64, return jglob[1024]."""
    gm = np.stack([np.asarray(resA_results[c]["gmax"]) for c in range(NCORES)])
    ab = np.stack(
        [np.asarray(resA_results[c]["accb"]).astype(np.float32) for c in range(NCORES)]
    )
    # [c, 128p, t, ...] -> row r = t*128 + p
    gm = gm.transpose(0, 2, 1, 3, 4).reshape(NCORES, B2, 2, 2048 // GROUP)
    ab = ab.transpose(0, 2, 1, 3).reshape(NCORES, B2, 2048)

    Mt = np.maximum(gm.max(axis=(0, 2, 3)), ab.max(axis=(0, 2)))  # [B2]
    eps = 2 * DELTA_R + np.abs(Mt) * 2.0 ** -8 + 1e-4
    thresh = Mt - eps

    dir_b = np.asarray(DIRECT_BS, dtype=np.int64)  # [NT, 2]
    act_b = np.asarray(ACT_BS, dtype=np.int64)  # [NT, 4]
    rows_list, pos_list = [], []
    c_i, r_i, s_i, g_i = np.nonzero(gm >= thresh[None, :, None, None])
    if len(c_i):
        base = (
            c_i.astype(np.int64) * QS
            + dir_b[r_i // 128, s_i] * 2048
            + g_i.astype(np.int64) * GROUP
        )
        pos = (base[:, None] + np.arange(GROUP, dtype=np.int64)[None, :]).reshape(-1)
        rows = np.repeat(r_i.astype(np.int64), GROUP)
        rows_list.append(rows)
        pos_list.append(pos)
    c_i, r_i, p_i = np.nonzero(ab >= thresh[None, :, None])
    if len(c_i):
        base = c_i.astype(np.int64) * QS + p_i.astype(np.int64)
        pos = (base[:, None] + act_b[r_i // 128] * 2048).reshape(-1)
        rows = np.repeat(r_i.astype(np.int64), act_b.shape[1])
        rows_list.append(rows)
        pos_list.append(pos)
    rows = np.concatenate(rows_list)
    pos = np.concatenate(pos_list)

    P64 = np.concatenate([p1, p2], axis=0).astype(np.float64)
    s = np.einsum("kd,kd->k", fq[pos].astype(np.float64), P64[rows])

    # first-occurrence argmax per row: sort by (row, pos), take first pos
    # attaining the row max
    order = np.lexsort((pos, rows))
    rows_s, pos_s, s_s = rows[order], pos[order], s[order]
    jglob = np.empty(B2, dtype=np.int64)
    starts = np.searchsorted(rows_s, np.arange(B2), side="left")
    ends = np.searchsorted(rows_s, np.arange(B2), side="right")
    for r in range(B2):
        sl = slice(starts[r], ends[r])
        sv = s_s[sl]
        jglob[r] = pos_s[sl][np.argmax(sv)]
    return jglob


def _prep_C_inmaps(p1, p2, nn, temp):
    def l2n(x):
        n = np.sqrt((x.astype(np.float64) ** 2).sum(axis=1, keepdims=True))
        return (x / np.maximum(n, 1e-12)).astype(np.float32)

    p1n = l2n(p1)
    p2n = l2n(p2)
    inv_t = np.float32(1.0) / np.float32(temp)
    p1s = (p1n * inv_t).astype(np.float32)
    p2s = (p2n * inv_t).astype(np.float32)
    nn1, nn2 = nn[:B], nn[B:]
    nn1_adj = ((nn1 - p1n) + p1n).astype(np.float32)
    nn2_adj = ((nn2 - p2n) + p2n).astype(np.float32)

    mats = [(nn1_adj, p2s), (p2s, nn1_adj), (nn2_adj, p1s), (p1s, nn2_adj)]
    in_maps = []
    for c in range(NCORES):
        m = c // 2
        i0 = (c % 2) * 2
        lhs, rhs = mats[m]
        lhsT = np.ascontiguousarray(lhs.T[:, i0 * 128 : (i0 + 2) * 128])
        rhsT = np.ascontiguousarray(rhs.T)
        lhsN = np.ascontiguousarray(
            lhs.reshape(4, 128, D)[i0 : i0 + 2].transpose(1, 0, 2)
        )
        rhsN = np.ascontiguousarray(
            rhs.reshape(4, 128, D)[i0 : i0 + 2].transpose(1, 0, 2)
        )
        in_maps.append({"lhsT": lhsT, "rhsT": rhsT, "lhsN": lhsN, "rhsN": rhsN})
    return in_maps


def kernel(projections_1, projections_2, feature_queue, temperature):
    from concourse.bass_utils import run_bass_kernel_spmd

    p1 = np.ascontiguousarray(projections_1, dtype=np.float32)
    p2 = np.ascontiguousarray(projections_2, dtype=np.float32)
    fq = np.ascontiguousarray(feature_queue, dtype=np.float32)

    ncA = _get_nc("A")
    resA = run_bass_kernel_spmd(ncA, _prep_A_inmaps(p1, p2, fq), core_ids=list(range(NCORES)))
    jglob = _host_top1(resA.results, p1, p2, fq)
    nn = fq[jglob]

    ncC = _get_nc("C")
    resC = run_bass_kernel_spmd(
        ncC, _prep_C_inmaps(p1, p2, nn, temperature), core_ids=list(range(NCORES))
    )
    loss = np.empty(4 * B, dtype=np.float32)
    for c in range(NCORES):
        out = np.asarray(resC.results[c]["loss"], dtype=np.float32)  # [128, 2]
        for j in range(2):
            rt = 2 * c + j
            loss[rt * 128 : (rt + 1) * 128] = out[:, j]
    return loss


# revision 17
# speedup vs baseline: 3.7568x; 1.0253x over previous
"""NNCLR forward loss kernel for 8x TRN2 NeuronCores.

Launch A (8 cores): fp32r sims of both projections (1024 rows) against the
core's queue shard (12288 rows). Scan split across engines: per tile of 128
rows x 6 blocks of 2048 cols, blocks b in {1,4} get a DVE grouped max (groups
of 32, fp32 from PSUM), blocks {0,2,3,5} get Act PSUM->SBUF bf16 convert +
DVE 2x-mode positional running max. Host screens all reported maxima within
an eps that covers fp32r + bf16 error, rescores candidate positions exactly
in fp64, and picks the global top-1 per row (first-occurrence ties).

Launch C (8 cores): each core computes 2 of the 16 [128, 512] logit row-tiles
(fp32r matmuls on host-prepped transposed operands), log-softmax + diagonal
extraction, returning its [2, 128] loss slice.
"""

import numpy as np

import concourse.bass as bass
import concourse.mybir as mybir
from concourse.tile import TileContext

import bass_rust as _br
import concourse.tile as _tile_mod


def _patched_drain_and_barrier(self, tick_clock, wait_clock):
    """Walrus here only allows 2 sem waits per instruction; split the
    Tile tail drain's wait list across extra drain instructions."""
    drain_inst = self.nc.sync.drain()
    wait_clock.add_sem_waits(
        drain_inst.ins, _br.ScopedClock({None: tick_clock.global_clock})
    )
    si = drain_inst.ins.sync_info
    if si is not None and si.on_wait and len(si.on_wait) > 1:
        waits = list(si.on_wait)
        drain_inst.ins.sync_info = _br.SyncInfo(on_wait=waits[:1], on_update=list(si.on_update))
        for i in range(1, len(waits)):
            extra = self.nc.sync.drain()
            extra.ins.sync_info = _br.SyncInfo(on_wait=waits[i : i + 1], on_update=[])
    self.nc.all_engine_barrier()
    assert self.sems is not None
    popped = self.nc._tile_sem_poison_stack.pop()
    assert popped is self._sem_poison
    self.nc.clear_and_free_semaphores(list(self.sems.allocated().values()))
    self.nc.all_engine_barrier()


_tile_mod.TileContext._drain_and_barrier = _patched_drain_and_barrier


def _split_multi_waits(nc):
    """This walrus build allows only one sync-wait per instruction; hoist
    extra waits onto NOPs inserted just before, on the same engine."""
    n_split = 0
    for f in nc.m.functions:
        for bb in f.blocks:
            il = bb.instructions
            i = 0
            while i < len(il):
                inst = il[i]
                si = inst.sync_info
                if si is not None and si.on_wait and len(si.on_wait) > 1:
                    waits = list(si.on_wait)
                    nops = []
                    for w in waits[:-1]:
                        nop = mybir.InstNoOp(
                            name=f"waitsplit-{nc.next_id()}",
                            engine=inst.engine,
                            ins=[],
                            outs=[],
                            sync_info=_br.SyncInfo(on_wait=[w], on_update=[]),
                        )
                        nc.register_instruction(nop, overwrite=True)
                        nops.append(nop)
                    inst.sync_info = _br.SyncInfo(
                        on_wait=[waits[-1]], on_update=list(si.on_update)
                    )
                    il[i:i] = nops
                    i += len(nops)
                    n_split += 1
                i += 1
    return n_split


F32 = mybir.dt.float32
F32R = mybir.dt.float32r
BF16 = mybir.dt.bfloat16
AF = mybir.ActivationFunctionType
AX = mybir.AxisListType

B = 512
D = 256
B2 = 2 * B  # 1024 combined rows (p1 then p2)
NCORES = 8
Q_FULL = 98304
QS = Q_FULL // NCORES  # 12288
NT = B2 // 128  # 8 row tiles
NB = QS // 2048  # 6 col blocks of 2048
# Per tile t: 3 blocks get a DVE grouped-max (fp32 from PSUM, groups of 32);
# the other 3 are converted to bf16 by Act and DMA'd out verbatim (exact
# positions for the host). Alternating patterns keep each block round at
# ~4 direct + ~4 act, well under the tensor round time on both engines.
DIRECT_BS = [
    [1, 3, 5] if t in (1, 3, 5) else [0, 2, 4] for t in range(NT)
]
ACT_BS = [[b for b in range(NB) if b not in DIRECT_BS[t]] for t in range(NT)]
GROUP = 32
# fp32r abs error bound: measured 7.5e-4 max (||p||~16 rows); 3x margin
DELTA_R = 2.5e-3
# fp8 (e4m3) sims: per-product RMS err ~5.1% -> sims err sigma ~0.051 abs;
# screening eps uses ~4.3 sigma per side
DELTA_8 = 0.22
FP8 = False  # use fp8e4 DoubleRow matmuls in launch A
Q8_SCALE = 8.0  # pre-scale on q before fp8 cast (host); undone on gm/ab


def build_nc_A():
    nc = bass.Bass(num_devices=NCORES, debug=False)
    pT = nc.declare_dram_parameter("pT", [D, B2], F32R, isOutput=False)
    qT = nc.declare_dram_parameter("qT", [D, QS], F32R, isOutput=False)
    gmax_out = nc.declare_dram_parameter(
        "gmax", [128, NT, 3, 2048 // GROUP], F32, isOutput=True
    )
    accb_out = nc.declare_dram_parameter("accb", [128, NT, 3, 2048], BF16, isOutput=True)

    with TileContext(nc) as tc:
        with (
            tc.tile_pool(name="persist", bufs=1) as pp,
            tc.tile_pool(name="qstream", bufs=3) as qp,
            tc.tile_pool(name="bconv", bufs=3) as bp,
            tc.tile_pool(name="psumA", bufs=2, space="PSUM") as psA,
        ):
            pT_sb = pp.tile([128, 2, B2], F32R)
            pT3 = pT.ap().rearrange("(k p) b -> p k b", p=128)
            # tiny first pieces on the scalar queue unblock warmup + matmul 0
            nc.scalar.dma_start(pT_sb[:, 0, 0:512], pT3[:, 0, 0:512])
            nc.scalar.dma_start(pT_sb[:, 1, 0:128], pT3[:, 1, 0:128])
            nc.scalar.dma_start(pT_sb[:, 0, 512:B2], pT3[:, 0, 512:B2])
            nc.scalar.dma_start(pT_sb[:, 1, 128:B2], pT3[:, 1, 128:B2])
            qT3 = qT.ap().rearrange("(k p) q -> p k q", p=128)
            gmax_sb = pp.tile([128, NT, 3, 2048 // GROUP], F32)

            q_tiles = []
            for b in range(NB):
                qt = qp.tile([128, 2, 2048], F32R, tag="q", name=f"q_{b}")
                if b == 0:
                    # first block in 4 chunk-DMAs so matmul 0 starts ASAP
                    for c4 in range(4):
                        nc.sync.dma_start(
                            qt[:, :, c4 * 512 : (c4 + 1) * 512],
                            qT3[:, :, c4 * 512 : (c4 + 1) * 512],
                        )
                else:
                    nc.sync.dma_start(qt[:], qT3[:, :, b * 2048 : (b + 1) * 2048])
                q_tiles.append(qt)

            for b in range(NB):
                qt = q_tiles[b]
                for t in range(NT):
                    ps = psA.tile([128, 2048], F32, tag="blk", name=f"ps_{t}_{b}")
                    for k in range(2):
                        for c4 in range(4):
                            nc.tensor.matmul(
                                ps[:, c4 * 512 : (c4 + 1) * 512],
                                pT_sb[:, k, t * 128 : (t + 1) * 128],
                                qt[:, k, c4 * 512 : (c4 + 1) * 512],
                                start=(k == 0),
                                stop=(k == 1),
                            )
                    if b in DIRECT_BS[t]:
                        slot = DIRECT_BS[t].index(b)
                        nc.vector.reduce_max(
                            gmax_sb[:, t, slot, :],
                            ps.rearrange("p (g k) -> p g k", k=GROUP),
                            axis=AX.X,
                        )
                        if b == DIRECT_BS[t][-1]:
                            nc.sync.dma_start(
                                gmax_out.ap()[:, t, :, :], gmax_sb[:, t, :, :]
                            )
                    else:
                        slot = ACT_BS[t].index(b)
                        bc = bp.tile([128, 2048], BF16, tag="bc", name=f"bc_{t}_{b}")
                        nc.scalar.copy(bc[:], ps[:])
                        nc.sync.dma_start(accb_out.ap()[:, t, slot, :], bc[:])

    _split_multi_waits(nc)
    return nc


def build_nc_C():
    nc = bass.Bass(num_devices=NCORES, debug=False)
    lhsT = nc.declare_dram_parameter("lhsT", [D, 256], F32R, isOutput=False)
    rhsT = nc.declare_dram_parameter("rhsT", [D, B], F32R, isOutput=False)
    lhsN = nc.declare_dram_parameter("lhsN", [128, 2, D], F32, isOutput=False)
    rhsN = nc.declare_dram_parameter("rhsN", [128, 2, D], F32, isOutput=False)
    loss_out = nc.declare_dram_parameter("loss", [128, 2], F32, isOutput=True)

    with TileContext(nc) as tc:
        with (
            tc.tile_pool(name="persist", bufs=1) as pp,
            tc.tile_pool(name="scr", bufs=2) as sp,
            tc.tile_pool(name="psumC", bufs=2, space="PSUM") as psC,
        ):
            rhsT3 = rhsT.ap().rearrange("(k p) c -> p k c", p=128)
            lhsT_sb = pp.tile([128, 2, 256], F32R)
            rhsT_sb = pp.tile([128, 2, B], F32R)
            # k=0 halves first (unblock the first matmul), spread across the
            # two HWDGE-capable queues (sync + scalar)
            nc.sync.dma_start(rhsT_sb[:, 0, :], rhsT3[:, 0, :])
            nc.scalar.dma_start(lhsT_sb[:], lhsT.ap().rearrange("(k p) c -> p k c", p=128))
            nc.sync.dma_start(rhsT_sb[:, 1, :], rhsT3[:, 1, :])
            lhsN_sb = pp.tile([128, 2, D], F32)
            nc.scalar.dma_start(lhsN_sb[:], lhsN.ap())
            rhsN_sb = pp.tile([128, 2, D], F32)
            nc.sync.dma_start(rhsN_sb[:], rhsN.ap())

            M = pp.tile([128, 2], F32)
            negM = pp.tile([128, 2], F32)
            S = pp.tile([128, 2], F32)
            dmul = pp.tile([128, 2, D], F32)
            dg = pp.tile([128, 2], F32)
            for j in range(2):
                psc = psC.tile([128, B], F32, tag="psc", name=f"psc_{j}")
                for k in range(2):
                    nc.tensor.matmul(
                        psc[:],
                        lhsT_sb[:, k, j * 128 : (j + 1) * 128],
                        rhsT_sb[:, k, :],
                        start=(k == 0),
                        stop=(k == 1),
                    )
                nc.vector.reduce_max(M[:, j : j + 1], psc[:], axis=AX.X)
                nc.vector.tensor_scalar_mul(negM[:, j : j + 1], M[:, j : j + 1], -1.0)
                escr = sp.tile([128, B], F32, tag="escr", name=f"escr_{j}")
                nc.scalar.activation(
                    escr[:], psc[:], AF.Exp,
                    bias=negM[:, j : j + 1], scale=1.0,
                    accum_out=S[:, j : j + 1],
                )
                # diag dot products overlap with the matmul/softmax chain
                nc.vector.tensor_mul(dmul[:, j, :], lhsN_sb[:, j, :], rhsN_sb[:, j, :])
                nc.vector.reduce_sum(dg[:, j : j + 1], dmul[:, j, :], axis=AX.X)
            lnS = pp.tile([128, 2], F32)
            nc.scalar.activation(lnS[:], S[:], AF.Ln)
            lossT = pp.tile([128, 2], F32)
            nc.vector.tensor_add(lossT[:], lnS[:], M[:])
            nc.vector.tensor_sub(lossT[:], lossT[:], dg[:])
            nc.sync.dma_start(loss_out.ap(), lossT[:])

    _split_multi_waits(nc)
    return nc


_CACHE = {}


def _get_nc(which):
    if which not in _CACHE:
        _CACHE[which] = build_nc_A() if which == "A" else build_nc_C()
    return _CACHE[which]


def _prep_A_inmaps(p1, p2, fq):
    pT_full = np.ascontiguousarray(np.concatenate([p1, p2], axis=0).T)
    return [
        {"pT": pT_full, "qT": np.ascontiguousarray(fq[c * QS : (c + 1) * QS].T)}
        for c in range(NCORES)
    ]


def _host_top1(resA_results, p1, p2, fq):
    """Screen device maxima, rescore candidates in fp64, return jglob[1024]."""
    gm = np.stack([np.asarray(resA_results[c]["gmax"]) for c in range(NCORES)])
    ab = np.stack(
        [np.asarray(resA_results[c]["accb"]).astype(np.float32) for c in range(NCORES)]
    )
    # [c, 128p, t, ...] -> row r = t*128 + p
    gm = gm.transpose(0, 2, 1, 3, 4).reshape(NCORES, B2, 3, 2048 // GROUP)
    ab = ab.transpose(0, 2, 1, 3, 4).reshape(NCORES, B2, 3, 2048)

    Mt = np.maximum(gm.max(axis=(0, 2, 3)), ab.max(axis=(0, 2, 3)))  # [B2]
    eps = 2 * DELTA_R + np.abs(Mt) * 2.0 ** -8 + 1e-4
    thresh = Mt - eps

    dir_b = np.asarray(DIRECT_BS, dtype=np.int64)  # [NT, 3]
    act_b = np.asarray(ACT_BS, dtype=np.int64)  # [NT, 3]
    rows_list, pos_list = [], []
    c_i, r_i, s_i, g_i = np.nonzero(gm >= thresh[None, :, None, None])
    if len(c_i):
        base = (
            c_i.astype(np.int64) * QS
            + dir_b[r_i // 128, s_i] * 2048
            + g_i.astype(np.int64) * GROUP
        )
        pos = (base[:, None] + np.arange(GROUP, dtype=np.int64)[None, :]).reshape(-1)
        rows = np.repeat(r_i.astype(np.int64), GROUP)
        rows_list.append(rows)
        pos_list.append(pos)
    c_i, r_i, s_i, p_i = np.nonzero(ab >= thresh[None, :, None, None])
    if len(c_i):
        pos = (
            c_i.astype(np.int64) * QS
            + act_b[r_i // 128, s_i] * 2048
            + p_i.astype(np.int64)
        )
        rows = r_i.astype(np.int64)
        rows_list.append(rows)
        pos_list.append(pos)
    rows = np.concatenate(rows_list)
    pos = np.concatenate(pos_list)

    P64 = np.concatenate([p1, p2], axis=0).astype(np.float64)
    s = np.einsum("kd,kd->k", fq[pos].astype(np.float64), P64[rows])

    # first-occurrence argmax per row: sort by (row, pos), take first pos
    # attaining the row max
    order = np.lexsort((pos, rows))
    rows_s, pos_s, s_s = rows[order], pos[order], s[order]
    jglob = np.empty(B2, dtype=np.int64)
    starts = np.searchsorted(rows_s, np.arange(B2), side="left")
    ends = np.searchsorted(rows_s, np.arange(B2), side="right")
    for r in range(B2):
        sl = slice(starts[r], ends[r])
        sv = s_s[sl]
        jglob[r] = pos_s[sl][np.argmax(sv)]
    return jglob


def _prep_C_inmaps(p1, p2, nn, temp):
    def l2n(x):
        n = np.sqrt((x.astype(np.float64) ** 2).sum(axis=1, keepdims=True))
        return (x / np.maximum(n, 1e-12)).astype(np.float32)

    p1n = l2n(p1)
    p2n = l2n(p2)
    inv_t = np.float32(1.0) / np.float32(temp)
    p1s = (p1n * inv_t).astype(np.float32)
    p2s = (p2n * inv_t).astype(np.float32)
    nn1, nn2 = nn[:B], nn[B:]
    nn1_adj = ((nn1 - p1n) + p1n).astype(np.float32)
    nn2_adj = ((nn2 - p2n) + p2n).astype(np.float32)

    mats = [(nn1_adj, p2s), (p2s, nn1_adj), (nn2_adj, p1s), (p1s, nn2_adj)]
    in_maps = []
    for c in range(NCORES):
        m = c // 2
        i0 = (c % 2) * 2
        lhs, rhs = mats[m]
        lhsT = np.ascontiguousarray(lhs.T[:, i0 * 128 : (i0 + 2) * 128])
        rhsT = np.ascontiguousarray(rhs.T)
        lhsN = np.ascontiguousarray(
            lhs.reshape(4, 128, D)[i0 : i0 + 2].transpose(1, 0, 2)
        )
        rhsN = np.ascontiguousarray(
            rhs.reshape(4, 128, D)[i0 : i0 + 2].transpose(1, 0, 2)
        )
        in_maps.append({"lhsT": lhsT, "rhsT": rhsT, "lhsN": lhsN, "rhsN": rhsN})
    return in_maps


def kernel(projections_1, projections_2, feature_queue, temperature):
    from concourse.bass_utils import run_bass_kernel_spmd

    p1 = np.ascontiguousarray(projections_1, dtype=np.float32)
    p2 = np.ascontiguousarray(projections_2, dtype=np.float32)
    fq = np.ascontiguousarray(feature_queue, dtype=np.float32)

    ncA = _get_nc("A")
    resA = run_bass_kernel_spmd(ncA, _prep_A_inmaps(p1, p2, fq), core_ids=list(range(NCORES)))
    jglob = _host_top1(resA.results, p1, p2, fq)
    nn = fq[jglob]

    ncC = _get_nc("C")
    resC = run_bass_kernel_spmd(
        ncC, _prep_C_inmaps(p1, p2, nn, temperature), core_ids=list(range(NCORES))
    )
    loss = np.empty(4 * B, dtype=np.float32)
    for c in range(NCORES):
        out = np.asarray(resC.results[c]["loss"], dtype=np.float32)  # [128, 2]
        for j in range(2):
            rt = 2 * c + j
            loss[rt * 128 : (rt + 1) * 128] = out[:, j]
    return loss


# revision 24
# speedup vs baseline: 4.5795x; 1.2190x over previous
"""NNCLR forward loss kernel for 8x TRN2 NeuronCores.

Launch A (8 cores): fp32r sims of both projections (1024 rows) against the
core's queue shard (12288 rows). Scan split across engines: per tile of 128
rows x 6 blocks of 2048 cols, blocks b in {1,4} get a DVE grouped max (groups
of 32, fp32 from PSUM), blocks {0,2,3,5} get Act PSUM->SBUF bf16 convert +
DVE 2x-mode positional running max. Host screens all reported maxima within
an eps that covers fp32r + bf16 error, rescores candidate positions exactly
in fp64, and picks the global top-1 per row (first-occurrence ties).

Launch C (8 cores): each core computes 2 of the 16 [128, 512] logit row-tiles
(fp32r matmuls on host-prepped transposed operands), log-softmax + diagonal
extraction, returning its [2, 128] loss slice.
"""

import numpy as np

import concourse.bass as bass
import concourse.mybir as mybir
from concourse.tile import TileContext

import bass_rust as _br
import concourse.tile as _tile_mod


def _patched_drain_and_barrier(self, tick_clock, wait_clock):
    """Walrus here only allows 2 sem waits per instruction; split the
    Tile tail drain's wait list across extra drain instructions."""
    drain_inst = self.nc.sync.drain()
    wait_clock.add_sem_waits(
        drain_inst.ins, _br.ScopedClock({None: tick_clock.global_clock})
    )
    si = drain_inst.ins.sync_info
    if si is not None and si.on_wait and len(si.on_wait) > 1:
        waits = list(si.on_wait)
        drain_inst.ins.sync_info = _br.SyncInfo(on_wait=waits[:1], on_update=list(si.on_update))
        for i in range(1, len(waits)):
            extra = self.nc.sync.drain()
            extra.ins.sync_info = _br.SyncInfo(on_wait=waits[i : i + 1], on_update=[])
    self.nc.all_engine_barrier()
    assert self.sems is not None
    popped = self.nc._tile_sem_poison_stack.pop()
    assert popped is self._sem_poison
    self.nc.clear_and_free_semaphores(list(self.sems.allocated().values()))
    self.nc.all_engine_barrier()


_tile_mod.TileContext._drain_and_barrier = _patched_drain_and_barrier


def _split_multi_waits(nc):
    """This walrus build allows only one sync-wait per instruction; hoist
    extra waits onto NOPs inserted just before, on the same engine."""
    n_split = 0
    for f in nc.m.functions:
        for bb in f.blocks:
            il = bb.instructions
            i = 0
            while i < len(il):
                inst = il[i]
                si = inst.sync_info
                if si is not None and si.on_wait and len(si.on_wait) > 1:
                    waits = list(si.on_wait)
                    nops = []
                    for w in waits[:-1]:
                        nop = mybir.InstNoOp(
                            name=f"waitsplit-{nc.next_id()}",
                            engine=inst.engine,
                            ins=[],
                            outs=[],
                            sync_info=_br.SyncInfo(on_wait=[w], on_update=[]),
                        )
                        nc.register_instruction(nop, overwrite=True)
                        nops.append(nop)
                    inst.sync_info = _br.SyncInfo(
                        on_wait=[waits[-1]], on_update=list(si.on_update)
                    )
                    il[i:i] = nops
                    i += len(nops)
                    n_split += 1
                i += 1
    return n_split


F32 = mybir.dt.float32
F32R = mybir.dt.float32r
BF16 = mybir.dt.bfloat16
AF = mybir.ActivationFunctionType
AX = mybir.AxisListType

B = 512
D = 256
B2 = 2 * B  # 1024 combined rows (p1 then p2)
NCORES = 8
Q_FULL = 98304
QS = Q_FULL // NCORES  # 12288
NT = B2 // 128  # 8 row tiles
NB = QS // 2048  # 6 col blocks of 2048
# Per tile t: 3 blocks get a DVE grouped-max (fp32 from PSUM, groups of 32);
# the other 3 are converted to bf16 by Act and DMA'd out verbatim (exact
# positions for the host). Alternating patterns keep each block round at
# ~4 direct + ~4 act, well under the tensor round time on both engines.
DIRECT_BS = [
    [1, 3, 5] if t in (1, 3, 5) else [0, 2, 4] for t in range(NT)
]
ACT_BS = [[b for b in range(NB) if b not in DIRECT_BS[t]] for t in range(NT)]
GROUP = 32
# fp32r abs error bound: measured 7.5e-4 max (||p||~16 rows); 3x margin
DELTA_R = 2.5e-3
# fp8 (e4m3) sims: per-product RMS err ~5.1% -> sims err sigma ~0.051 abs;
# screening eps uses ~4.3 sigma per side
DELTA_8 = 0.22
FP8 = True  # use fp8e4 DoubleRow matmuls in launch A
Q8_SCALE = 8.0  # pre-scale on q before fp8 cast (host); undone on gm/ab


def build_nc_A():
    in_dt = mybir.dt.float8e4 if FP8 else F32R
    nc = bass.Bass(num_devices=NCORES, debug=False)
    pT = nc.declare_dram_parameter("pT", [D, B2], in_dt, isOutput=False)
    qT = nc.declare_dram_parameter("qT", [D, QS], in_dt, isOutput=False)
    gmax_out = nc.declare_dram_parameter(
        "gmax", [128, NT, 3, 2048 // GROUP], F32, isOutput=True
    )
    accb_out = nc.declare_dram_parameter("accb", [128, NT, 3, 2048], BF16, isOutput=True)

    with TileContext(nc) as tc:
        with (
            tc.tile_pool(name="persist", bufs=1) as pp,
            tc.tile_pool(name="qstream", bufs=3) as qp,
            tc.tile_pool(name="bconv", bufs=3) as bp,
            tc.tile_pool(name="psumA", bufs=2, space="PSUM") as psA,
        ):
            pT_sb = pp.tile([128, 2, B2], in_dt)
            pT3 = pT.ap().rearrange("(k p) b -> p k b", p=128)
            # tiny first pieces on the scalar queue unblock warmup + matmul 0
            nc.scalar.dma_start(pT_sb[:, 0, 0:512], pT3[:, 0, 0:512])
            nc.scalar.dma_start(pT_sb[:, 1, 0:128], pT3[:, 1, 0:128])
            nc.scalar.dma_start(pT_sb[:, 0, 512:B2], pT3[:, 0, 512:B2])
            nc.scalar.dma_start(pT_sb[:, 1, 128:B2], pT3[:, 1, 128:B2])
            qT3 = qT.ap().rearrange("(k p) q -> p k q", p=128)
            gmax_sb = pp.tile([128, NT, 3, 2048 // GROUP], F32)

            q_tiles = []
            for b in range(NB):
                qt = qp.tile([128, 2, 2048], in_dt, tag="q", name=f"q_{b}")
                if b == 0:
                    # first block in 4 chunk-DMAs so matmul 0 starts ASAP
                    for c4 in range(4):
                        nc.sync.dma_start(
                            qt[:, :, c4 * 512 : (c4 + 1) * 512],
                            qT3[:, :, c4 * 512 : (c4 + 1) * 512],
                        )
                else:
                    nc.sync.dma_start(qt[:], qT3[:, :, b * 2048 : (b + 1) * 2048])
                q_tiles.append(qt)

            for b in range(NB):
                qt = q_tiles[b]
                for t in range(NT):
                    ps = psA.tile([128, 2048], F32, tag="blk", name=f"ps_{t}_{b}")
                    if FP8:
                        # DoubleRow: both K halves contracted in one pass
                        for c4 in range(4):
                            nc.tensor.matmul(
                                ps[:, c4 * 512 : (c4 + 1) * 512],
                                pT_sb[:, :, t * 128 : (t + 1) * 128],
                                qt[:, :, c4 * 512 : (c4 + 1) * 512],
                                start=True,
                                stop=True,
                                perf_mode=mybir.MatmulPerfMode.DoubleRow,
                            )
                    else:
                        for k in range(2):
                            for c4 in range(4):
                                nc.tensor.matmul(
                                    ps[:, c4 * 512 : (c4 + 1) * 512],
                                    pT_sb[:, k, t * 128 : (t + 1) * 128],
                                    qt[:, k, c4 * 512 : (c4 + 1) * 512],
                                    start=(k == 0),
                                    stop=(k == 1),
                                )
                    if b in DIRECT_BS[t]:
                        slot = DIRECT_BS[t].index(b)
                        nc.vector.reduce_max(
                            gmax_sb[:, t, slot, :],
                            ps.rearrange("p (g k) -> p g k", k=GROUP),
                            axis=AX.X,
                        )
                        if b == DIRECT_BS[t][-1]:
                            nc.sync.dma_start(
                                gmax_out.ap()[:, t, :, :], gmax_sb[:, t, :, :]
                            )
                    else:
                        slot = ACT_BS[t].index(b)
                        bc = bp.tile([128, 2048], BF16, tag="bc", name=f"bc_{t}_{b}")
                        nc.scalar.copy(bc[:], ps[:])
                        nc.sync.dma_start(accb_out.ap()[:, t, slot, :], bc[:])

    _split_multi_waits(nc)
    return nc


def build_nc_C():
    nc = bass.Bass(num_devices=NCORES, debug=False)
    lhsT = nc.declare_dram_parameter("lhsT", [D, 256], F32R, isOutput=False)
    rhsT = nc.declare_dram_parameter("rhsT", [D, B], F32R, isOutput=False)
    lhsN = nc.declare_dram_parameter("lhsN", [128, 2, D], F32, isOutput=False)
    rhsN = nc.declare_dram_parameter("rhsN", [128, 2, D], F32, isOutput=False)
    loss_out = nc.declare_dram_parameter("loss", [128, 2], F32, isOutput=True)

    with TileContext(nc) as tc:
        with (
            tc.tile_pool(name="persist", bufs=1) as pp,
            tc.tile_pool(name="scr", bufs=2) as sp,
            tc.tile_pool(name="psumC", bufs=2, space="PSUM") as psC,
        ):
            rhsT3 = rhsT.ap().rearrange("(k p) c -> p k c", p=128)
            lhsT_sb = pp.tile([128, 2, 256], F32R)
            rhsT_sb = pp.tile([128, 2, B], F32R)
            # k=0 halves first (unblock the first matmul), spread across the
            # two HWDGE-capable queues (sync + scalar)
            nc.sync.dma_start(rhsT_sb[:, 0, :], rhsT3[:, 0, :])
            nc.scalar.dma_start(lhsT_sb[:], lhsT.ap().rearrange("(k p) c -> p k c", p=128))
            nc.sync.dma_start(rhsT_sb[:, 1, :], rhsT3[:, 1, :])
            lhsN_sb = pp.tile([128, 2, D], F32)
            nc.scalar.dma_start(lhsN_sb[:], lhsN.ap())
            rhsN_sb = pp.tile([128, 2, D], F32)
            nc.sync.dma_start(rhsN_sb[:], rhsN.ap())

            M = pp.tile([128, 2], F32)
            negM = pp.tile([128, 2], F32)
            S = pp.tile([128, 2], F32)
            dmul = pp.tile([128, 2, D], F32)
            dg = pp.tile([128, 2], F32)
            for j in range(2):
                psc = psC.tile([128, B], F32, tag="psc", name=f"psc_{j}")
                for k in range(2):
                    nc.tensor.matmul(
                        psc[:],
                        lhsT_sb[:, k, j * 128 : (j + 1) * 128],
                        rhsT_sb[:, k, :],
                        start=(k == 0),
                        stop=(k == 1),
                    )
                nc.vector.reduce_max(M[:, j : j + 1], psc[:], axis=AX.X)
                nc.vector.tensor_scalar_mul(negM[:, j : j + 1], M[:, j : j + 1], -1.0)
                escr = sp.tile([128, B], F32, tag="escr", name=f"escr_{j}")
                nc.scalar.activation(
                    escr[:], psc[:], AF.Exp,
                    bias=negM[:, j : j + 1], scale=1.0,
                    accum_out=S[:, j : j + 1],
                )
                # diag dot products overlap with the matmul/softmax chain
                nc.vector.tensor_mul(dmul[:, j, :], lhsN_sb[:, j, :], rhsN_sb[:, j, :])
                nc.vector.reduce_sum(dg[:, j : j + 1], dmul[:, j, :], axis=AX.X)
            lnS = pp.tile([128, 2], F32)
            nc.scalar.activation(lnS[:], S[:], AF.Ln)
            lossT = pp.tile([128, 2], F32)
            nc.vector.tensor_add(lossT[:], lnS[:], M[:])
            nc.vector.tensor_sub(lossT[:], lossT[:], dg[:])
            nc.sync.dma_start(loss_out.ap(), lossT[:])

    _split_multi_waits(nc)
    return nc


_CACHE = {}


def _get_nc(which):
    if which not in _CACHE:
        _CACHE[which] = build_nc_A() if which == "A" else build_nc_C()
    return _CACHE[which]


def _prep_A_inmaps(p1, p2, fq):
    pT_full = np.ascontiguousarray(np.concatenate([p1, p2], axis=0).T)
    if FP8:
        import ml_dtypes

        pT_full = pT_full.astype(ml_dtypes.float8_e4m3)
        return [
            {
                "pT": pT_full,
                "qT": np.ascontiguousarray(
                    (fq[c * QS : (c + 1) * QS].T * np.float32(Q8_SCALE))
                ).astype(ml_dtypes.float8_e4m3),
            }
            for c in range(NCORES)
        ]
    return [
        {"pT": pT_full, "qT": np.ascontiguousarray(fq[c * QS : (c + 1) * QS].T)}
        for c in range(NCORES)
    ]


def _host_top1(resA_results, p1, p2, fq):
    """Screen device maxima, rescore candidates in fp64, return jglob[1024]."""
    gm = np.stack([np.asarray(resA_results[c]["gmax"]) for c in range(NCORES)])
    ab = np.stack(
        [np.asarray(resA_results[c]["accb"]).astype(np.float32) for c in range(NCORES)]
    )
    # [c, 128p, t, ...] -> row r = t*128 + p
    gm = gm.transpose(0, 2, 1, 3, 4).reshape(NCORES, B2, 3, 2048 // GROUP)
    ab = ab.transpose(0, 2, 1, 3, 4).reshape(NCORES, B2, 3, 2048)
    if FP8:
        gm /= np.float32(Q8_SCALE)
        ab /= np.float32(Q8_SCALE)

    delta = DELTA_8 if FP8 else DELTA_R
    Mt = np.maximum(gm.max(axis=(0, 2, 3)), ab.max(axis=(0, 2, 3)))  # [B2]
    eps = 2 * delta + np.abs(Mt) * 2.0 ** -8 + 1e-4
    thresh = Mt - eps

    dir_b = np.asarray(DIRECT_BS, dtype=np.int64)  # [NT, 3]
    act_b = np.asarray(ACT_BS, dtype=np.int64)  # [NT, 3]
    rows_list, pos_list = [], []
    c_i, r_i, s_i, g_i = np.nonzero(gm >= thresh[None, :, None, None])
    if len(c_i):
        base = (
            c_i.astype(np.int64) * QS
            + dir_b[r_i // 128, s_i] * 2048
            + g_i.astype(np.int64) * GROUP
        )
        pos = (base[:, None] + np.arange(GROUP, dtype=np.int64)[None, :]).reshape(-1)
        rows = np.repeat(r_i.astype(np.int64), GROUP)
        rows_list.append(rows)
        pos_list.append(pos)
    c_i, r_i, s_i, p_i = np.nonzero(ab >= thresh[None, :, None, None])
    if len(c_i):
        pos = (
            c_i.astype(np.int64) * QS
            + act_b[r_i // 128, s_i] * 2048
            + p_i.astype(np.int64)
        )
        rows = r_i.astype(np.int64)
        rows_list.append(rows)
        pos_list.append(pos)
    rows = np.concatenate(rows_list)
    pos = np.concatenate(pos_list)

    P64 = np.concatenate([p1, p2], axis=0).astype(np.float64)
    s = np.einsum("kd,kd->k", fq[pos].astype(np.float64), P64[rows])

    # first-occurrence argmax per row: sort by (row, pos), take first pos
    # attaining the row max
    order = np.lexsort((pos, rows))
    rows_s, pos_s, s_s = rows[order], pos[order], s[order]
    jglob = np.empty(B2, dtype=np.int64)
    starts = np.searchsorted(rows_s, np.arange(B2), side="left")
    ends = np.searchsorted(rows_s, np.arange(B2), side="right")
    for r in range(B2):
        sl = slice(starts[r], ends[r])
        sv = s_s[sl]
        jglob[r] = pos_s[sl][np.argmax(sv)]
    return jglob


def _prep_C_inmaps(p1, p2, nn, temp):
    def l2n(x):
        n = np.sqrt((x.astype(np.float64) ** 2).sum(axis=1, keepdims=True))
        return (x / np.maximum(n, 1e-12)).astype(np.float32)

    p1n = l2n(p1)
    p2n = l2n(p2)
    inv_t = np.float32(1.0) / np.float32(temp)
    p1s = (p1n * inv_t).astype(np.float32)
    p2s = (p2n * inv_t).astype(np.float32)
    nn1, nn2 = nn[:B], nn[B:]
    nn1_adj = ((nn1 - p1n) + p1n).astype(np.float32)
    nn2_adj = ((nn2 - p2n) + p2n).astype(np.float32)

    mats = [(nn1_adj, p2s), (p2s, nn1_adj), (nn2_adj, p1s), (p1s, nn2_adj)]
    in_maps = []
    for c in range(NCORES):
        m = c // 2
        i0 = (c % 2) * 2
        lhs, rhs = mats[m]
        lhsT = np.ascontiguousarray(lhs.T[:, i0 * 128 : (i0 + 2) * 128])
        rhsT = np.ascontiguousarray(rhs.T)
        lhsN = np.ascontiguousarray(
            lhs.reshape(4, 128, D)[i0 : i0 + 2].transpose(1, 0, 2)
        )
        rhsN = np.ascontiguousarray(
            rhs.reshape(4, 128, D)[i0 : i0 + 2].transpose(1, 0, 2)
        )
        in_maps.append({"lhsT": lhsT, "rhsT": rhsT, "lhsN": lhsN, "rhsN": rhsN})
    return in_maps


def kernel(projections_1, projections_2, feature_queue, temperature):
    from concourse.bass_utils import run_bass_kernel_spmd

    p1 = np.ascontiguousarray(projections_1, dtype=np.float32)
    p2 = np.ascontiguousarray(projections_2, dtype=np.float32)
    fq = np.ascontiguousarray(feature_queue, dtype=np.float32)

    ncA = _get_nc("A")
    resA = run_bass_kernel_spmd(ncA, _prep_A_inmaps(p1, p2, fq), core_ids=list(range(NCORES)))
    jglob = _host_top1(resA.results, p1, p2, fq)
    nn = fq[jglob]

    ncC = _get_nc("C")
    resC = run_bass_kernel_spmd(
        ncC, _prep_C_inmaps(p1, p2, nn, temperature), core_ids=list(range(NCORES))
    )
    loss = np.empty(4 * B, dtype=np.float32)
    for c in range(NCORES):
        out = np.asarray(resC.results[c]["loss"], dtype=np.float32)  # [128, 2]
        for j in range(2):
            rt = 2 * c + j
            loss[rt * 128 : (rt + 1) * 128] = out[:, j]
    return loss
